# revision 1
# baseline (speedup 1.0000x reference)
"""NeighborAttentionLayer Trainium2 kernel (8-core data-parallel SPMD).

Strategy
--------
Data-parallel over the batch dim B=64: each of the 8 NeuronCores runs the
full transformer layer for 8 batches (1024 tokens). No collectives.

Host-side prep (numpy, not on HW):
  * weights transposed to [in_features, out_features], cast to bf16, and
    pre-tiled into per-tile-contiguous blocks so every weight DMA is one
    fully contiguous transfer
  * 1/sqrt(head_dim) folded into the q projection
  * q/k out-features permuted into a head-pair-interleaved order so every
    head's 320 features map onto 128-partition tiles as 128+128+64 slices
    (the 64-wide slices of a head pair share one tile at base 0 / 64)
  * x shard passed both natural fp32 (residual) and transposed bf16 (matmul)

The learned distance-bias MLP adds a per-query bias broadcast over keys;
softmax over keys is invariant to it, so it is skipped. The key-padding
mask is all-ones per the problem spec (fill=ones); a non-trivial mask is
applied multiplicatively on the exp'd scores.

All matmuls run in bf16 with fp32 PSUM accumulation. Softmax / layernorm /
residual arithmetic is fp32.
"""

import numpy as np
import ml_dtypes

# ---- problem constants (hardcoded per contract) ----
B, K, D, H, DFF = 64, 128, 2560, 8, 1024
HD = D // H                    # 320
EPS = 1e-5
NCORES = 8
BL = B // NCORES               # 8 batches per core
TOK = BL * K                   # 1024 tokens per core
P = 128
DT = D // P                    # 20 d-tiles
FT = DFF // P                  # 8 dff-tiles
CH = 512                       # matmul moving-dim chunk (psum bank limit)
NHALF = 2                      # token halves for attention SBUF pressure
THALF = TOK // NHALF           # 512 tokens per half
BHALF = BL // NHALF            # 4 batches per half
QKT = 2 * DT                   # 40 q+k feature tiles


def _qk_perm():
    """Head-pair interleaved feature order for q (and k) projections."""
    perm = []
    for p in range(H // 2):
        h0, h1 = 2 * p, 2 * p + 1
        perm.extend(range(HD * h0, HD * h0 + 256))         # tiles 5p+0, 5p+1
        perm.extend(range(HD * h0 + 256, HD * h0 + 320))   # tile 5p+2 lo
        perm.extend(range(HD * h1 + 256, HD * h1 + 320))   # tile 5p+2 hi
        perm.extend(range(HD * h1, HD * h1 + 256))         # tiles 5p+3, 5p+4
    return np.array(perm)


def _score_ktiles(h):
    """(tile, row0, row1) triples (within the 20 q-tiles) contracting head h."""
    p = h // 2
    if h % 2 == 0:
        return [(5 * p + 0, 0, 128), (5 * p + 1, 0, 128), (5 * p + 2, 0, 64)]
    return [(5 * p + 3, 0, 128), (5 * p + 4, 0, 128), (5 * p + 2, 64, 128)]


def _ao_segments():
    """Per d-tile (real feature order) segments for attn@V:
    list over tiles of [(head, d0, d1, psum_base), ...]."""
    segs = [[] for _ in range(DT)]
    for h in range(H):
        d = HD * h
        end = HD * (h + 1)
        while d < end:
            nxt = min(end, (d // P + 1) * P)
            segs[d // P].append((h, d, nxt, d % P))
            d = nxt
    return segs


def _tileize(wT, chunk):
    """[Kin, N] -> [N/chunk, 128, Kin/128, chunk] contiguous blocks."""
    kin, n = wT.shape
    ko = kin // P
    return np.ascontiguousarray(
        wT.reshape(ko, P, n // chunk, chunk).transpose(2, 1, 0, 3))


def build_core_program(use_qk_bias, use_v_bias, use_out_bias, use_b1, use_b2,
                       ln1_affine, ln2_affine, use_mask):
    import concourse.bass as bass
    import concourse.bacc as bacc
    import concourse.mybir as mybir
    import concourse.tile as tile
    from concourse.masks import make_identity

    F32 = mybir.dt.float32
    BF16 = mybir.dt.bfloat16

    nc = bacc.Bacc()
    dp = nc.declare_dram_parameter
    xT = dp("xT", [NHALF, P, DT, THALF], BF16, isOutput=False)
    x_nat = dp("x", [TOK, D], F32, isOutput=False)
    qk_wT = dp("qk_wT", [QKT, P, DT, P], BF16, isOutput=False)
    v_wT = dp("v_wT", [D // CH, P, DT, CH], BF16, isOutput=False)
    out_wT = dp("out_wT", [D // CH, P, DT, CH], BF16, isOutput=False)
    w1T = dp("w1T", [FT, P, DT, P], BF16, isOutput=False)
    w2T = dp("w2T", [D // CH, P, FT, CH], BF16, isOutput=False)
    qk_b = dp("qk_b", [2 * D], F32, isOutput=False) if use_qk_bias else None
    v_b = dp("v_b", [D], F32, isOutput=False) if use_v_bias else None
    out_b = dp("out_b", [D], F32, isOutput=False) if use_out_bias else None
    b1 = dp("b1", [DFF], F32, isOutput=False) if use_b1 else None
    b2 = dp("b2", [D], F32, isOutput=False) if use_b2 else None
    ln1_g = dp("ln1_g", [D], F32, isOutput=False) if ln1_affine else None
    ln1_b = dp("ln1_b", [D], F32, isOutput=False) if ln1_affine else None
    ln2_g = dp("ln2_g", [D], F32, isOutput=False) if ln2_affine else None
    ln2_b = dp("ln2_b", [D], F32, isOutput=False) if ln2_affine else None
    mask_in = dp("mask", [BL, K], F32, isOutput=False) if use_mask else None
    out = dp("out", [TOK, D], F32, isOutput=True)

    x1_dram = nc.dram_tensor("x1_scratch", [TOK, D], F32)
    aoT_dram = nc.dram_tensor("aoT_scratch", [BL, P, DT, P], BF16)

    Exp = mybir.ActivationFunctionType.Exp
    Relu = mybir.ActivationFunctionType.Relu
    Sqrt = mybir.ActivationFunctionType.Sqrt
    Copy = mybir.ActivationFunctionType.Copy
    Ident = mybir.ActivationFunctionType.Identity
    AX = mybir.AxisListType.X
    OP = mybir.AluOpType

    def bcast_dram(ap, n_part=P):
        return bass.AP(tensor=ap.tensor, offset=ap.offset,
                       ap=[[0, n_part]] + list(ap.ap))

    ao_segs = _ao_segments()

    with tile.TileContext(nc) as tc:
        with (
            tc.tile_pool(name="consts", bufs=1) as consts,
        ):
            id_bf = consts.tile([P, P], BF16)
            make_identity(nc, id_bf)
            id_f32 = consts.tile([P, P], F32)
            make_identity(nc, id_f32)
            eps_sb = consts.tile([P, 1], F32)
            nc.vector.memset(eps_sb, EPS)

            # first out_proj weight chunk, preloaded so phase C starts hot
            wo_first = consts.tile([P, DT, CH], BF16)
            nc.sync.dma_start(out=wo_first, in_=out_wT[0])

            qkb_sb = None
            if use_qk_bias:
                qkb_sb = consts.tile([P, QKT], F32)
                nc.sync.dma_start(out=qkb_sb,
                                  in_=qk_b[:].rearrange("(t p) -> p t", p=P))
            vb_sb = None
            if use_v_bias:
                vb_sb = consts.tile([P, D], F32)
                nc.gpsimd.dma_start(out=vb_sb, in_=bcast_dram(v_b[:]))
            outb_sb = None
            if use_out_bias:
                outb_sb = consts.tile([P, D], F32)
                nc.gpsimd.dma_start(out=outb_sb, in_=bcast_dram(out_b[:]))
            b1_sb = None
            if use_b1:
                b1_sb = consts.tile([P, FT], F32)
                nc.sync.dma_start(out=b1_sb,
                                  in_=b1[:].rearrange("(t p) -> p t", p=P))
            b2_sb = None
            if use_b2:
                b2_sb = consts.tile([P, D], F32)
                nc.gpsimd.dma_start(out=b2_sb, in_=bcast_dram(b2[:]))
            ln1g_sb = ln1b_sb = ln2g_sb = ln2b_sb = None
            if ln1_affine:
                ln1g_sb = consts.tile([P, D], F32)
                nc.gpsimd.dma_start(out=ln1g_sb, in_=bcast_dram(ln1_g[:]))
                ln1b_sb = consts.tile([P, D], F32)
                nc.gpsimd.dma_start(out=ln1b_sb, in_=bcast_dram(ln1_b[:]))
            if ln2_affine:
                ln2g_sb = consts.tile([P, D], F32)
                nc.gpsimd.dma_start(out=ln2g_sb, in_=bcast_dram(ln2_g[:]))
                ln2b_sb = consts.tile([P, D], F32)
                nc.gpsimd.dma_start(out=ln2b_sb, in_=bcast_dram(ln2_b[:]))
            mask_sb = None
            if use_mask:
                mask_sb = consts.tile([P, BL, K], F32)
                nc.gpsimd.dma_start(
                    out=mask_sb, in_=bcast_dram(mask_in[:, :]))

            # ======== attention: both halves share one set of buffers ========
            with (
                tc.tile_pool(name="attn_sb", bufs=1) as asb,
                tc.tile_pool(name="aw", bufs=3) as aw,
                tc.tile_pool(name="bt", bufs=2) as bt,
            ):
                xT_sb = asb.tile([P, DT, THALF], BF16)
                v_sb = asb.tile([P, BHALF, D], BF16)
                qkT_sb = asb.tile([P, QKT, THALF], BF16)

                for half in range(NHALF):
                    nc.sync.dma_start(out=xT_sb, in_=xT[half])

                    with tc.tile_pool(name=f"aps{half}", bufs=4,
                                      space="PSUM") as aps:
                        # V projection: natural [tok, vfeat]
                        for c in range(D // CH):
                            wv = aw.tile([P, DT, CH], BF16, tag="wv")
                            nc.sync.dma_start(out=wv, in_=v_wT[c])
                            for t in range(BHALF):
                                ps = aps.tile([P, CH], F32, tag="ps_a")
                                for k in range(DT):
                                    nc.tensor.matmul(
                                        ps, xT_sb[:, k, t * P:(t + 1) * P],
                                        wv[:, k, :],
                                        start=(k == 0), stop=(k == DT - 1))
                                if use_v_bias:
                                    nc.vector.tensor_add(
                                        out=v_sb[:, t, c * CH:(c + 1) * CH],
                                        in0=ps,
                                        in1=vb_sb[:, c * CH:(c + 1) * CH])
                                else:
                                    nc.vector.tensor_copy(
                                        out=v_sb[:, t, c * CH:(c + 1) * CH],
                                        in_=ps)

                        # Q/K projection: transposed [feat, tok]
                        for jt in range(QKT):
                            wq = aw.tile([P, DT, P], BF16, tag="wq")
                            nc.sync.dma_start(out=wq, in_=qk_wT[jt])
                            ps = aps.tile([P, CH], F32, tag="ps_a")
                            for k in range(DT):
                                nc.tensor.matmul(ps, wq[:, k, :], xT_sb[:, k, :],
                                                 start=(k == 0),
                                                 stop=(k == DT - 1))
                            if use_qk_bias:
                                nc.scalar.activation(
                                    out=qkT_sb[:, jt, :], in_=ps, func=Ident,
                                    bias=qkb_sb[:, jt:jt + 1], scale=1.0)
                            else:
                                nc.scalar.activation(out=qkT_sb[:, jt, :],
                                                     in_=ps, func=Copy)

                    # attention per batch: scores -> transposes -> attn@V,
                    # each stage contiguous on PE so no mid-stream waits
                    with (
                        tc.tile_pool(name=f"sps{half}", bufs=4,
                                     space="PSUM") as sps,
                        tc.tile_pool(name=f"tps{half}", bufs=2,
                                     space="PSUM") as tps,
                        tc.tile_pool(name=f"ops{half}", bufs=2,
                                     space="PSUM") as ops,
                    ):
                        for bi in range(BHALF):
                            b = half * BHALF + bi
                            csl = slice(bi * P, (bi + 1) * P)
                            attn = bt.tile([P, H, P], BF16, tag="attn")
                            negmax = bt.tile([P, H], F32, tag="negmax")
                            esum = bt.tile([P, H], F32, tag="esum")
                            rinv = bt.tile([P, H], F32, tag="rinv")
                            attnT = bt.tile([P, H, P], BF16, tag="attnT")
                            scs = []
                            for h in range(H):
                                sc = sps.tile([P, P], F32, tag="sc")
                                scs.append(sc)
                                kts = _score_ktiles(h)
                                for i, (t, r0, r1) in enumerate(kts):
                                    nc.tensor.matmul(
                                        sc, qkT_sb[r0:r1, t, csl],
                                        qkT_sb[r0:r1, DT + t, csl],
                                        start=(i == 0), stop=(i == len(kts) - 1))
                                nc.vector.tensor_reduce(
                                    out=negmax[:, h:h + 1], in_=sc, axis=AX,
                                    op=OP.max, negate=True)
                                nc.scalar.activation(
                                    out=attn[:, h, :], in_=sc, func=Exp,
                                    bias=negmax[:, h:h + 1], scale=1.0,
                                    accum_out=esum[:, h:h + 1])
                                if use_mask:
                                    nc.vector.tensor_mul(
                                        out=attn[:, h, :], in0=attn[:, h, :],
                                        in1=mask_sb[:, b, :])
                                    nc.vector.tensor_reduce(
                                        out=esum[:, h:h + 1], in_=attn[:, h, :],
                                        axis=AX, op=OP.add)
                                nc.vector.reciprocal(out=rinv[:, h:h + 1],
                                                     in_=esum[:, h:h + 1])
                                nc.vector.tensor_scalar_mul(
                                    out=attn[:, h, :], in0=attn[:, h, :],
                                    scalar1=rinv[:, h:h + 1])
                            for h in range(H):
                                tp = tps.tile([P, P], BF16, tag="tp")
                                nc.tensor.transpose(tp, attn[:, h, :], id_bf)
                                nc.vector.tensor_copy(out=attnT[:, h, :], in_=tp)
                            ao_stage = bt.tile([P, DT, P], BF16, tag="ao_stage")
                            for t in range(DT):
                                ao = ops.tile([P, P], F32, tag="ao")
                                for (h, d0, d1, base) in ao_segs[t]:
                                    w = d1 - d0
                                    nc.tensor.matmul(
                                        ao[base:base + w, :], v_sb[:, bi, d0:d1],
                                        attnT[:, h, :], start=True, stop=True,
                                        tile_position=((0, base) if base
                                                       else None))
                                nc.scalar.activation(out=ao_stage[:, t, :],
                                                     in_=ao, func=Copy)
                            nc.sync.dma_start(out=aoT_dram[b], in_=ao_stage)

            # ======== out_proj + residual + LN1 + FFN1, per token group ======
            NGRP = 2
            TPG = BL // NGRP          # tok-tiles per group
            GW = TPG * P              # tokens per group (512)
            with tc.tile_pool(name="hres", bufs=1) as hres:
                hT = hres.tile([P, FT, TOK], BF16)
                with (
                    tc.tile_pool(name="csb", bufs=2) as csb,
                    tc.tile_pool(name="cao", bufs=2) as cao,
                    tc.tile_pool(name="cw", bufs=2) as cw,
                    tc.tile_pool(name="cy", bufs=1) as cy,
                    tc.tile_pool(name="cx1t", bufs=1) as cx1t,
                    tc.tile_pool(name="dw", bufs=3) as dw,
                    tc.tile_pool(name="cps", bufs=4, space="PSUM") as cps,
                    tc.tile_pool(name="ctps", bufs=2, space="PSUM") as ctps,
                ):
                    for g in range(NGRP):
                        y_grp = cy.tile([P, TPG, D], F32, tag="y_grp")
                        stats_g = csb.tile([P, TPG, 5, 6], F32, tag="stats")
                        for c in range(D // CH):
                            if g == 0 and c == 0:
                                wo = wo_first
                            else:
                                wo = cw.tile([P, DT, CH], BF16, tag="wo")
                                nc.sync.dma_start(out=wo, in_=out_wT[c])
                            for ti in range(TPG):
                                tt = g * TPG + ti
                                aoT_t = cao.tile([P, DT, P], BF16, tag="aoT_t")
                                nc.sync.dma_start(out=aoT_t, in_=aoT_dram[tt])
                                ps = cps.tile([P, CH], F32, tag="ps")
                                for k in range(DT):
                                    nc.tensor.matmul(
                                        ps, aoT_t[:, k, :], wo[:, k, :],
                                        start=(k == 0), stop=(k == DT - 1))
                                if use_out_bias:
                                    nc.vector.tensor_add(
                                        out=ps, in0=ps,
                                        in1=outb_sb[:, c * CH:(c + 1) * CH])
                                xr = csb.tile([P, CH], F32, tag="xr")
                                nc.sync.dma_start(
                                    out=xr,
                                    in_=x_nat[tt * P:(tt + 1) * P,
                                              c * CH:(c + 1) * CH])
                                nc.vector.tensor_add(
                                    out=y_grp[:, ti, c * CH:(c + 1) * CH],
                                    in0=ps, in1=xr)
                                nc.vector.bn_stats(
                                    out=stats_g[:, ti, c, :],
                                    in_=y_grp[:, ti, c * CH:(c + 1) * CH])
                        # LN1 per tok-tile; x1 -> scratch (residual), x1T -> SBUF
                        x1T_grp = cx1t.tile([P, DT, GW], BF16, tag="x1T_grp")
                        for ti in range(TPG):
                            tt = g * TPG + ti
                            yt = y_grp[:, ti, :]
                            mv = csb.tile([P, 2], F32, tag="mv")
                            nc.vector.bn_aggr(out=mv, in_=stats_g[:, ti])
                            std = csb.tile([P, 1], F32, tag="std")
                            nc.scalar.activation(out=std, in_=mv[:, 1:2],
                                                 func=Sqrt, bias=eps_sb,
                                                 scale=1.0)
                            rstd = csb.tile([P, 1], F32, tag="rstd")
                            nc.vector.reciprocal(out=rstd, in_=std)
                            x1_t = csb.tile([P, D], F32, tag="x1t")
                            nc.vector.tensor_scalar(out=x1_t, in0=yt,
                                                    scalar1=mv[:, 0:1],
                                                    scalar2=rstd,
                                                    op0=OP.subtract, op1=OP.mult)
                            if ln1_affine:
                                nc.vector.tensor_mul(out=x1_t, in0=x1_t,
                                                     in1=ln1g_sb)
                                nc.vector.tensor_add(out=x1_t, in0=x1_t,
                                                     in1=ln1b_sb)
                            nc.sync.dma_start(
                                out=x1_dram[tt * P:(tt + 1) * P, :], in_=x1_t)
                            for k in range(DT):
                                tp = ctps.tile([P, P], F32, tag="tp_c")
                                nc.tensor.transpose(
                                    tp, x1_t[:, k * P:(k + 1) * P], id_f32)
                                nc.scalar.activation(
                                    out=x1T_grp[:, k, ti * P:(ti + 1) * P],
                                    in_=tp, func=Copy)
                        # FFN1 for this group's tokens (relu, output into hT)
                        for ft in range(FT):
                            w1 = dw.tile([P, DT, P], BF16, tag="w1")
                            nc.sync.dma_start(out=w1, in_=w1T[ft])
                            ps = cps.tile([P, CH], F32, tag="ps")
                            for k in range(DT):
                                nc.tensor.matmul(
                                    ps, w1[:, k, :], x1T_grp[:, k, :],
                                    start=(k == 0), stop=(k == DT - 1))
                            osl = slice(g * GW, (g + 1) * GW)
                            if use_b1:
                                nc.scalar.activation(
                                    out=hT[:, ft, osl], in_=ps, func=Relu,
                                    bias=b1_sb[:, ft:ft + 1], scale=1.0)
                            else:
                                nc.scalar.activation(out=hT[:, ft, osl],
                                                     in_=ps, func=Relu)

                # ======== FFN2 + residual + LN2, per token group ========
                with (
                    tc.tile_pool(name="esb", bufs=2) as esb,
                    tc.tile_pool(name="ey", bufs=1) as ey,
                    tc.tile_pool(name="ew", bufs=2) as ew,
                    tc.tile_pool(name="eps", bufs=4, space="PSUM") as epsp,
                ):
                    for g in range(NGRP):
                        y2 = ey.tile([P, TPG, D], F32, tag="y2")
                        stats_e = esb.tile([P, TPG, 5, 6], F32, tag="stats_e")
                        for c in range(D // CH):
                            w2c = ew.tile([P, FT, CH], BF16, tag="w2c")
                            nc.sync.dma_start(out=w2c, in_=w2T[c])
                            for ti in range(TPG):
                                tt = g * TPG + ti
                                ps = epsp.tile([P, CH], F32, tag="ps_e")
                                for k in range(FT):
                                    nc.tensor.matmul(
                                        ps, hT[:, k, tt * P:(tt + 1) * P],
                                        w2c[:, k, :],
                                        start=(k == 0), stop=(k == FT - 1))
                                if use_b2:
                                    nc.vector.tensor_add(
                                        out=ps, in0=ps,
                                        in1=b2_sb[:, c * CH:(c + 1) * CH])
                                xr = esb.tile([P, CH], F32, tag="xr_e")
                                nc.sync.dma_start(
                                    out=xr,
                                    in_=x1_dram[tt * P:(tt + 1) * P,
                                                c * CH:(c + 1) * CH])
                                nc.vector.tensor_add(
                                    out=y2[:, ti, c * CH:(c + 1) * CH],
                                    in0=ps, in1=xr)
                                nc.vector.bn_stats(
                                    out=stats_e[:, ti, c, :],
                                    in_=y2[:, ti, c * CH:(c + 1) * CH])
                        for ti in range(TPG):
                            tt = g * TPG + ti
                            mv = esb.tile([P, 2], F32, tag="mv_e")
                            nc.vector.bn_aggr(out=mv, in_=stats_e[:, ti])
                            std = esb.tile([P, 1], F32, tag="std_e")
                            nc.scalar.activation(out=std, in_=mv[:, 1:2],
                                                 func=Sqrt, bias=eps_sb,
                                                 scale=1.0)
                            rstd = esb.tile([P, 1], F32, tag="rstd_e")
                            nc.vector.reciprocal(out=rstd, in_=std)
                            o_t = esb.tile([P, D], F32, tag="o_t")
                            nc.vector.tensor_scalar(out=o_t, in0=y2[:, ti, :],
                                                    scalar1=mv[:, 0:1],
                                                    scalar2=rstd,
                                                    op0=OP.subtract,
                                                    op1=OP.mult)
                            if ln2_affine:
                                nc.vector.tensor_mul(out=o_t, in0=o_t,
                                                     in1=ln2g_sb)
                                nc.vector.tensor_add(out=o_t, in0=o_t,
                                                     in1=ln2b_sb)
                            nc.sync.dma_start(
                                out=out[tt * P:(tt + 1) * P, :], in_=o_t)

    nc.compile()
    return nc


def _prep_inputs(x, distances, mask, qkv_w, qkv_b, out_w, out_b,
                 bias_w1, bias_b1, bias_w2, bias_b2,
                 ffn_w1, ffn_b1, ffn_w2, ffn_b2,
                 ln1_g, ln1_b, ln2_g, ln2_b):
    """Host-side shard + weight formatting. Returns (flags, in_maps)."""
    bf16 = ml_dtypes.bfloat16
    perm = _qk_perm()

    q_w = qkv_w[0:D][perm] * np.float32(1.0 / np.sqrt(HD))
    k_w = qkv_w[D:2 * D][perm]
    v_w = qkv_w[2 * D:3 * D]
    qk_wT = _tileize(np.concatenate([q_w, k_w], axis=0).T.astype(bf16), P)
    v_wT = _tileize(v_w.T.astype(bf16), CH)
    out_wT = _tileize(out_w.T.astype(bf16), CH)
    w1T = _tileize(ffn_w1.T.astype(bf16), P)
    w2T = _tileize(ffn_w2.T.astype(bf16), CH)

    qk_b = np.concatenate([qkv_b[0:D][perm] * np.float32(1.0 / np.sqrt(HD)),
                           qkv_b[D:2 * D][perm]]).astype(np.float32)
    v_b = np.ascontiguousarray(qkv_b[2 * D:3 * D]).astype(np.float32)

    flags = dict(
        use_qk_bias=bool(np.any(qk_b != 0)),
        use_v_bias=bool(np.any(v_b != 0)),
        use_out_bias=bool(np.any(out_b != 0)),
        use_b1=bool(np.any(ffn_b1 != 0)),
        use_b2=bool(np.any(ffn_b2 != 0)),
        ln1_affine=not (np.all(ln1_g == 1) and np.all(ln1_b == 0)),
        ln2_affine=not (np.all(ln2_g == 1) and np.all(ln2_b == 0)),
        use_mask=not bool(np.all(mask)),
    )

    shared = {"qk_wT": qk_wT, "v_wT": v_wT, "out_wT": out_wT,
              "w1T": w1T, "w2T": w2T}
    if flags["use_qk_bias"]:
        shared["qk_b"] = qk_b
    if flags["use_v_bias"]:
        shared["v_b"] = v_b
    if flags["use_out_bias"]:
        shared["out_b"] = out_b.astype(np.float32)
    if flags["use_b1"]:
        shared["b1"] = ffn_b1.astype(np.float32)
    if flags["use_b2"]:
        shared["b2"] = ffn_b2.astype(np.float32)
    if flags["ln1_affine"]:
        shared["ln1_g"] = ln1_g.astype(np.float32)
        shared["ln1_b"] = ln1_b.astype(np.float32)
    if flags["ln2_affine"]:
        shared["ln2_g"] = ln2_g.astype(np.float32)
        shared["ln2_b"] = ln2_b.astype(np.float32)

    in_maps = []
    for c in range(NCORES):
        xc = np.ascontiguousarray(
            x[c * BL:(c + 1) * BL].reshape(TOK, D)).astype(np.float32)
        xcT = xc.T.astype(bf16)          # [D, TOK]
        xT_blocks = np.ascontiguousarray(
            xcT.reshape(DT, P, NHALF, THALF).transpose(2, 1, 0, 3))
        m = {"x": xc, "xT": xT_blocks, **shared}
        if flags["use_mask"]:
            m["mask"] = mask[c * BL:(c + 1) * BL].astype(np.float32)
        in_maps.append(m)
    return flags, in_maps


def run(trace=False, **inputs):
    """Build + run on 8 cores. Returns (output, BassKernelResults)."""
    from concourse.bass_utils import run_bass_kernel_spmd

    inputs = {k: np.asarray(v) for k, v in inputs.items()}
    flags, in_maps = _prep_inputs(**inputs)
    nc = build_core_program(**flags)
    res = run_bass_kernel_spmd(nc, in_maps, list(range(NCORES)), trace=trace)
    out = np.stack([np.asarray(res.results[c]["out"], dtype=np.float32)
                    for c in range(NCORES)])
    return out.reshape(B, K, D), res


def kernel(**inputs):
    out, _ = run(trace=False, **inputs)
    return out



# revision 13
# speedup vs baseline: 1.4260x; 1.4260x over previous
"""NeighborAttentionLayer Trainium2 kernel (8-core data-parallel SPMD).

Strategy
--------
Data-parallel over B=64: each NeuronCore runs the full layer for 8 batches
(1024 tokens). No collectives.

v2: fp8-e4m3 DoubleRow matmuls (2 k-tiles contracted per PE pass) for the
V projection, out_proj, and optionally the Q/K projections (QF8/KF8 set how
many of the 20 contraction tiles run fp8; the bf16 remainder accumulates
into the same PSUM with host-matched scales). Scores / attn@V / FFN stay
bf16 for accuracy. attn-out is kept transposed in SBUF as fp8 (no DRAM
round-trip). out_proj -> LN1 -> FFN1 -> FFN2 -> LN2 run as one fused
per-token-tile pipeline: residual adds on DVE from PSUM, LN applied by the
scalar engine (Identity with per-partition bias/scale). Softmax skips
max-subtraction (logits bounded; exp in fp32). Per-batch attention work is
interleaved with dense GEMM chains so the PE never idles on softmax.

Scale bookkeeping: fp8 operands carry power-of-2 scales (SX on x, W*S on
weights, AOS on attn-out). The out_proj PSUM is AOS*WOS-scaled; the
residual x ships pre-scaled and LN1's eps is (AOS*WOS)^2-scaled, so
normalization absorbs the whole scale exactly.
"""

import numpy as np
import ml_dtypes
from contextlib import ExitStack

# ---- problem constants (hardcoded per contract) ----
B, K, D, H, DFF = 64, 128, 2560, 8, 1024
HD = D // H                    # 320
EPS = 1e-5
NCORES = 8
BL = B // NCORES               # 8 batches per core
TOK = BL * K                   # 1024 tokens per core
P = 128
DT = D // P                    # 20 d-tiles
FT = DFF // P                  # 8 dff-tiles
CH = 512                       # matmul moving-dim chunk (psum bank limit)
NHALF = 2
THALF = TOK // NHALF           # 512 tokens per half
BHALF = BL // NHALF            # 4 batches per half
QKT = 2 * DT                   # 40 q+k feature tiles
NC_CH = D // CH                # 5 output chunks of 512

# ---- fp8 knobs ----
QF8 = 20      # leading k-tiles (of 20) of the Q projection contracted in fp8
KF8 = 0       # same for K projection
AOS = 16.0    # fp8 scale for attn-out


def _qk_perm():
    """Head-pair interleaved feature order for q (and k) projections."""
    perm = []
    for p in range(H // 2):
        h0, h1 = 2 * p, 2 * p + 1
        perm.extend(range(HD * h0, HD * h0 + 256))         # tiles 5p+0, 5p+1
        perm.extend(range(HD * h0 + 256, HD * h0 + 320))   # tile 5p+2 lo
        perm.extend(range(HD * h1 + 256, HD * h1 + 320))   # tile 5p+2 hi
        perm.extend(range(HD * h1, HD * h1 + 256))         # tiles 5p+3, 5p+4
    return np.array(perm)


def _score_ktiles(h):
    """(tile, row0, row1) triples (within the 20 q-tiles) contracting head h."""
    p = h // 2
    if h % 2 == 0:
        return [(5 * p + 0, 0, 128), (5 * p + 1, 0, 128), (5 * p + 2, 0, 64)]
    return [(5 * p + 3, 0, 128), (5 * p + 4, 0, 128), (5 * p + 2, 64, 128)]


def _ao_segments():
    """Per d-tile (real feature order) segments for attn@V:
    list over tiles of [(head, d0, d1, psum_base), ...]."""
    segs = [[] for _ in range(DT)]
    for h in range(H):
        d = HD * h
        end = HD * (h + 1)
        while d < end:
            nxt = min(end, (d // P + 1) * P)
            segs[d // P].append((h, d, nxt, d % P))
            d = nxt
    return segs


def _tileize(wT, chunk):
    """[Kin, N] -> [N/chunk, 128, Kin/128, chunk] contiguous blocks."""
    kin, n = wT.shape
    ko = kin // P
    return np.ascontiguousarray(
        wT.reshape(ko, P, n // chunk, chunk).transpose(2, 1, 0, 3))


def _po2(a, target=224.0):
    m = float(np.abs(a).max())
    if m == 0.0:
        return 1.0
    return float(2.0 ** np.floor(np.log2(target / m)))


def build_core_program(use_qk_bias, use_v_bias, use_out_bias, use_b1, use_b2,
                       ln1_affine, ln2_affine, use_mask,
                       sx, wqs, wks, wvs, wos):
    import concourse.bass as bass
    import concourse.bacc as bacc
    import concourse.mybir as mybir
    import concourse.tile as tile
    from concourse.masks import make_identity

    F32 = mybir.dt.float32
    BF16 = mybir.dt.bfloat16
    F8 = mybir.dt.float8e4
    DR = mybir.MatmulPerfMode.DoubleRow

    need_xtb = (QF8 < DT) or (KF8 < DT)
    need_qk8 = (QF8 > 0) or (KF8 > 0)
    rscale = AOS * wos            # out_proj psum / residual scale

    nc = bacc.Bacc()
    dp = nc.declare_dram_parameter
    xT8 = dp("xT8", [NHALF, P, DT, THALF], F8, isOutput=False)
    xTb = dp("xTb", [NHALF, P, DT, THALF], BF16, isOutput=False) \
        if need_xtb else None
    qk_w8 = dp("qk_w8", [QKT, P, DT, P], F8, isOutput=False) \
        if need_qk8 else None
    qk_wb = dp("qk_wb", [QKT, P, DT, P], BF16, isOutput=False) \
        if need_xtb else None
    v_w8 = dp("v_w8", [NC_CH, P, DT, CH], F8, isOutput=False)
    out_w8 = dp("out_w8", [NC_CH, P, DT, CH], F8, isOutput=False)
    w1T = dp("w1T", [FT, P, DT, P], BF16, isOutput=False)
    w2T = dp("w2T", [NC_CH, P, FT, CH], BF16, isOutput=False)
    x_res = dp("x_res", [TOK, D], F32, isOutput=False)
    qk_b = dp("qk_b", [2 * D], F32, isOutput=False) if use_qk_bias else None
    v_b = dp("v_b", [D], F32, isOutput=False) if use_v_bias else None
    out_b = dp("out_b", [D], F32, isOutput=False) if use_out_bias else None
    b1 = dp("b1", [DFF], F32, isOutput=False) if use_b1 else None
    b2 = dp("b2", [D], F32, isOutput=False) if use_b2 else None
    ln1_g = dp("ln1_g", [D], F32, isOutput=False) if ln1_affine else None
    ln1_b = dp("ln1_b", [D], F32, isOutput=False) if ln1_affine else None
    ln2_g = dp("ln2_g", [D], F32, isOutput=False) if ln2_affine else None
    ln2_b = dp("ln2_b", [D], F32, isOutput=False) if ln2_affine else None
    mask_in = dp("mask", [BL, K], F32, isOutput=False) if use_mask else None
    out = dp("out", [TOK, D], F32, isOutput=True)

    x1_dram = nc.dram_tensor("x1_scratch", [TOK, D], BF16)

    Exp = mybir.ActivationFunctionType.Exp
    Relu = mybir.ActivationFunctionType.Relu
    Sqrt = mybir.ActivationFunctionType.Sqrt
    Copy = mybir.ActivationFunctionType.Copy
    Ident = mybir.ActivationFunctionType.Identity
    AX = mybir.AxisListType.X
    OP = mybir.AluOpType

    q_evac = 1.0 / (sx * wqs * float(np.sqrt(HD)))
    k_evac = 1.0 / (sx * wks)
    v_evac = 1.0 / (sx * wvs)

    def bcast_dram(ap_, n_part=P):
        return bass.AP(tensor=ap_.tensor, offset=ap_.offset,
                       ap=[[0, n_part]] + list(ap_.ap))

    ao_segs = _ao_segments()

    with tile.TileContext(nc) as tc, ExitStack() as st:
        consts = st.enter_context(tc.tile_pool(name="consts", bufs=1))
        persist = st.enter_context(tc.tile_pool(name="persist", bufs=1))
        # PSUM: 8 banks total, slots are bank-aligned.
        gps = st.enter_context(tc.tile_pool(name="gps", bufs=3, space="PSUM"))
        sps = st.enter_context(tc.tile_pool(name="sps", bufs=2, space="PSUM"))
        tps = st.enter_context(tc.tile_pool(name="tps", bufs=2, space="PSUM"))
        ops = st.enter_context(tc.tile_pool(name="ops", bufs=1, space="PSUM"))

        id_bf = consts.tile([P, P], BF16)
        make_identity(nc, id_bf)
        eps1_sb = consts.tile([P, 1], F32)
        nc.vector.memset(eps1_sb, rscale * rscale * EPS)
        eps2_sb = consts.tile([P, 1], F32)
        nc.vector.memset(eps2_sb, EPS)

        qkb_sb = None
        if use_qk_bias:
            qkb_sb = consts.tile([P, QKT], F32)
            nc.sync.dma_start(out=qkb_sb,
                              in_=qk_b[:].rearrange("(t p) -> p t", p=P))
        vb_sb = None
        if use_v_bias:
            vb_sb = consts.tile([P, D], F32)
            nc.gpsimd.dma_start(out=vb_sb, in_=bcast_dram(v_b[:]))
        outb_sb = None
        if use_out_bias:
            outb_sb = consts.tile([P, D], F32)
            nc.gpsimd.dma_start(out=outb_sb, in_=bcast_dram(out_b[:]))
        b1_sb = None
        if use_b1:
            b1_sb = consts.tile([P, FT], F32)
            nc.sync.dma_start(out=b1_sb,
                              in_=b1[:].rearrange("(t p) -> p t", p=P))
        b2_sb = None
        if use_b2:
            b2_sb = consts.tile([P, D], F32)
            nc.gpsimd.dma_start(out=b2_sb, in_=bcast_dram(b2[:]))
        ln1g_sb = ln1b_sb = ln2g_sb = ln2b_sb = None
        if ln1_affine:
            ln1g_sb = consts.tile([P, D], F32)
            nc.gpsimd.dma_start(out=ln1g_sb, in_=bcast_dram(ln1_g[:]))
            ln1b_sb = consts.tile([P, D], F32)
            nc.gpsimd.dma_start(out=ln1b_sb, in_=bcast_dram(ln1_b[:]))
        if ln2_affine:
            ln2g_sb = consts.tile([P, D], F32)
            nc.gpsimd.dma_start(out=ln2g_sb, in_=bcast_dram(ln2_g[:]))
            ln2b_sb = consts.tile([P, D], F32)
            nc.gpsimd.dma_start(out=ln2b_sb, in_=bcast_dram(ln2_b[:]))
        mask_sb = None
        if use_mask:
            mask_sb = consts.tile([P, BL, K], F32)
            nc.gpsimd.dma_start(out=mask_sb, in_=bcast_dram(mask_in[:, :]))

        # attn-out, transposed, fp8, SBUF-resident across phases
        aoT_sb = persist.tile([P, DT, TOK], F8)

        # -------------- attention pools (right side, close early) -------
        st_attn = ExitStack()
        attn_outer = st_attn.enter_context(
            tc.tile_pool(name="attn_outer", bufs=1, side="right"))
        bt = st_attn.enter_context(
            tc.tile_pool(name="bt", bufs=5, side="right"))
        btT = st_attn.enter_context(
            tc.tile_pool(name="btT", bufs=2, side="right"))

        qk_st = ExitStack()
        qkT_pool = qk_st.enter_context(
            tc.tile_pool(name="qkT", bufs=1, side="right"))
        qkT_sb = qkT_pool.tile([P, QKT, THALF], BF16, tag="qkT")

        proj_st = ExitStack()
        app = proj_st.enter_context(
            tc.tile_pool(name="attn_proj", bufs=1, side="right"))
        aw = proj_st.enter_context(
            tc.tile_pool(name="aw", bufs=2, side="right"))

        def dma_x8(half):
            x8 = app.tile([P, DT, THALF], F8, tag="x8", bufs=2)
            for s in range(4):
                nc.sync.dma_start(out=x8[:, 5 * s:5 * (s + 1), :],
                                  in_=xT8[half, :, 5 * s:5 * (s + 1), :])
            return x8

        def dma_xb(half):
            if not need_xtb:
                return None
            xb = app.tile([P, DT, THALF], BF16, tag="xb", bufs=1)
            for s in range(4):
                nc.sync.dma_start(out=xb[:, 5 * s:5 * (s + 1), :],
                                  in_=xTb[half, :, 5 * s:5 * (s + 1), :])
            return xb

        def emit_v_proj(xv, v_sb, c_range):
            for c in c_range:
                wv = aw.tile([P, DT, CH], F8, tag="wv")
                nc.sync.dma_start(out=wv, in_=v_w8[c])
                for t in range(BHALF):
                    ps = gps.tile([P, CH], F32, tag="ps")
                    for kp in range(DT // 2):
                        nc.tensor.matmul(
                            ps, xv[:, 2 * kp:2 * kp + 2, t * P:(t + 1) * P],
                            wv[:, 2 * kp:2 * kp + 2, :],
                            start=(kp == 0), stop=(kp == DT // 2 - 1),
                            perf_mode=DR)
                    osl = v_sb[:, t, c * CH:(c + 1) * CH]
                    nc.scalar.activation(out=osl, in_=ps, func=Copy,
                                         scale=v_evac)
                    if use_v_bias:
                        nc.vector.tensor_add(
                            out=osl, in0=osl,
                            in1=vb_sb[:, c * CH:(c + 1) * CH])

        def emit_qk_proj(x8, xb, j_range):
            for jt in j_range:
                f8n = QF8 if jt < DT else KF8
                evac = q_evac if jt < DT else k_evac
                w8t = wbt = None
                if f8n > 0:
                    w8t = aw.tile([P, f8n, P], F8, tag=f"w8_{f8n}")
                    nc.sync.dma_start(out=w8t, in_=qk_w8[jt][:, 0:f8n, :])
                if f8n < DT:
                    wbt = aw.tile([P, DT - f8n, P], BF16, tag=f"wb_{f8n}")
                    nc.sync.dma_start(out=wbt, in_=qk_wb[jt][:, f8n:DT, :])
                ps = gps.tile([P, CH], F32, tag="ps")
                n_mm = f8n // 2 + (DT - f8n)
                i = 0
                for kp in range(f8n // 2):
                    nc.tensor.matmul(
                        ps, w8t[:, 2 * kp:2 * kp + 2, :],
                        x8[:, 2 * kp:2 * kp + 2, :],
                        start=(i == 0), stop=(i == n_mm - 1), perf_mode=DR)
                    i += 1
                for k in range(f8n, DT):
                    nc.tensor.matmul(
                        ps, wbt[:, k - f8n, :], xb[:, k, :],
                        start=(i == 0), stop=(i == n_mm - 1))
                    i += 1
                if use_qk_bias:
                    nc.scalar.activation(out=qkT_sb[:, jt, :], in_=ps,
                                         func=Ident,
                                         bias=qkb_sb[:, jt:jt + 1],
                                         scale=evac)
                else:
                    nc.scalar.activation(out=qkT_sb[:, jt, :], in_=ps,
                                         func=Copy, scale=evac)

        def emit_scores_softmax(b):
            bi = b % BHALF
            csl = slice(bi * P, (bi + 1) * P)
            attn = bt.tile([P, H, P], BF16, tag="attn")
            esum = bt.tile([P, H], F32, tag="esum")
            rinv = bt.tile([P, H], F32, tag="rinv")
            for h in range(H):
                sc = sps.tile([P, P], F32, tag="sc")
                kts = _score_ktiles(h)
                for i, (t, r0, r1) in enumerate(kts):
                    nc.tensor.matmul(
                        sc, qkT_sb[r0:r1, t, csl],
                        qkT_sb[r0:r1, DT + t, csl],
                        start=(i == 0), stop=(i == len(kts) - 1))
                nc.scalar.activation(out=attn[:, h, :], in_=sc, func=Exp,
                                     accum_out=esum[:, h:h + 1])
                if use_mask:
                    nc.vector.tensor_mul(
                        out=attn[:, h, :], in0=attn[:, h, :],
                        in1=mask_sb[:, b, :])
                    nc.vector.tensor_reduce(
                        out=esum[:, h:h + 1], in_=attn[:, h, :],
                        axis=AX, op=OP.add)
                nc.vector.reciprocal(out=rinv[:, h:h + 1],
                                     in_=esum[:, h:h + 1])
                nc.vector.tensor_scalar_mul(
                    out=attn[:, h, :], in0=attn[:, h, :],
                    scalar1=rinv[:, h:h + 1])
            return attn

        def emit_tr_ao(b, attn, v_sb):
            bi = b % BHALF
            attnT = btT.tile([P, H, P], BF16, tag="attnT")
            for h in range(H):
                tp = tps.tile([P, P], BF16, tag="tp")
                nc.tensor.transpose(tp, attn[:, h, :], id_bf)
                nc.vector.tensor_copy(out=attnT[:, h, :], in_=tp)
            for t in range(DT):
                ao = ops.tile([P, P], F32, tag="ao")
                for (h, d0, d1, base) in ao_segs[t]:
                    w = d1 - d0
                    nc.tensor.matmul(
                        ao[base:base + w, :], v_sb[:, bi, d0:d1],
                        attnT[:, h, :], start=True, stop=True,
                        tile_position=((0, base) if base else None))
                nc.scalar.activation(
                    out=aoT_sb[:, t, b * P:(b + 1) * P], in_=ao,
                    func=Copy, scale=AOS)

        # ---- phase A emission: projections + batches 0..3 interleaved ----
        # tr/ao of batches 0..3 interleave with the half-1 Q/K GEMM chains
        # (not the V chains: v_sb is single-buffered, so its half-1 writers
        # must be emitted after every half-0 reader). Scores of batches 4..7
        # run right after QK h1 so the qkT pool can be released early.
        x8_0 = dma_x8(0)
        v_sb0 = attn_outer.tile([P, BHALF, D], BF16, tag="v")
        emit_v_proj(x8_0, v_sb0, range(NC_CH))
        xb_0 = dma_xb(0)
        emit_qk_proj(x8_0, xb_0, range(QKT))
        attns = {}
        for b in range(BHALF):
            attns[b] = emit_scores_softmax(b)
        x8_1 = dma_x8(1)
        xb_1 = dma_xb(1)
        for b in range(BHALF):
            emit_tr_ao(b, attns[b], v_sb0)
            emit_qk_proj(x8_1, xb_1, range(10 * b, 10 * (b + 1)))
        for b in range(BHALF, BL):
            attns[b] = emit_scores_softmax(b)
        v_sb1 = attn_outer.tile([P, BHALF, D], BF16, tag="v")
        emit_v_proj(x8_1, v_sb1, range(NC_CH))
        proj_st.close()      # frees xT8/xTb/aw SBUF
        qk_st.close()        # frees qkT SBUF (scores all emitted)

        # ------------- fused phase C pools (left side) -------------
        y_pool = st.enter_context(tc.tile_pool(name="y_pool", bufs=2))
        x1b_pool = st.enter_context(tc.tile_pool(name="x1b", bufs=2))
        xr_pool = st.enter_context(tc.tile_pool(name="xr", bufs=4))
        stat_pool = st.enter_context(tc.tile_pool(name="stat", bufs=2))
        wo_pool = st.enter_context(tc.tile_pool(name="wo", bufs=1))
        hT_pool = st.enter_context(tc.tile_pool(name="hT", bufs=1))
        hT = hT_pool.tile([P, FT, TOK], BF16)
        c2x = ExitStack()
        x1T_pool = c2x.enter_context(tc.tile_pool(name="x1T", bufs=1))
        x1T = x1T_pool.tile([P, DT, TOK], BF16)

        wo_tiles = []
        for c in range(NC_CH):
            wo = wo_pool.tile([P, DT, CH], F8, tag=f"wo{c}")
            nc.sync.dma_start(out=wo, in_=out_w8[c])
            wo_tiles.append(wo)

        def emit_ln(y_t, stats, eps_sb, g_sb, b_sb, out_t, affine):
            """LN over [P, D] given per-chunk bn stats; writes out_t."""
            mv = stat_pool.tile([P, 2], F32, tag="mv")
            nc.vector.bn_aggr(out=mv, in_=stats)
            std = stat_pool.tile([P, 1], F32, tag="std")
            nc.scalar.activation(out=std, in_=mv[:, 1:2], func=Sqrt,
                                 bias=eps_sb, scale=1.0)
            rstd = stat_pool.tile([P, 1], F32, tag="rstd")
            nc.vector.reciprocal(out=rstd, in_=std)
            nmr = stat_pool.tile([P, 1], F32, tag="nmr")
            nc.vector.tensor_scalar(out=nmr, in0=mv[:, 0:1], scalar1=rstd,
                                    scalar2=-1.0, op0=OP.mult, op1=OP.mult)
            nc.scalar.activation(out=out_t, in_=y_t, func=Ident,
                                 bias=nmr, scale=rstd)
            if affine:
                nc.vector.tensor_mul(out=out_t, in0=out_t, in1=g_sb)
                nc.vector.tensor_add(out=out_t, in0=out_t, in1=b_sb)

        def emit_op_tile(ti):
            """out_proj + residual + LN1 for token tile ti -> x1_dram."""
            tt = slice(ti * P, (ti + 1) * P)
            y_t = y_pool.tile([P, D], F32, tag="y")
            stats = stat_pool.tile([P, NC_CH, 6], F32, tag="stats")
            for c in range(NC_CH):
                xr = xr_pool.tile([P, CH], F32, tag="xr")
                nc.sync.dma_start(
                    out=xr, in_=x_res[ti * P:(ti + 1) * P,
                                      c * CH:(c + 1) * CH])
                ps = gps.tile([P, CH], F32, tag="ps")
                for kp in range(DT // 2):
                    nc.tensor.matmul(
                        ps, aoT_sb[:, 2 * kp:2 * kp + 2, tt],
                        wo_tiles[c][:, 2 * kp:2 * kp + 2, :],
                        start=(kp == 0), stop=(kp == DT // 2 - 1),
                        perf_mode=DR)
                csl = slice(c * CH, (c + 1) * CH)
                nc.vector.tensor_add(out=y_t[:, csl], in0=ps, in1=xr)
                if use_out_bias:
                    nc.vector.tensor_add(out=y_t[:, csl], in0=y_t[:, csl],
                                         in1=outb_sb[:, csl])
                nc.vector.bn_stats(out=stats[:, c, :], in_=y_t[:, csl])
            x1b = x1b_pool.tile([P, D], BF16, tag="x1b")
            emit_ln(y_t, stats, eps1_sb, ln1g_sb, ln1b_sb, x1b, ln1_affine)
            nc.sync.dma_start(out=x1_dram[ti * P:(ti + 1) * P, :], in_=x1b)
            # transpose x1 into x1T straight from SBUF
            for k in range(DT):
                tp = tps.tile([P, P], BF16, tag="tp")
                nc.tensor.transpose(tp, x1b[:, k * P:(k + 1) * P], id_bf)
                nc.vector.tensor_copy(
                    out=x1T[:, k, ti * P:(ti + 1) * P], in_=tp)

        def emit_ffn1(g):
            gsl = slice(g * THALF, (g + 1) * THALF)
            for ft in range(FT):
                w1 = w1_pool.tile([P, DT, P], BF16, tag="w1")
                nc.sync.dma_start(out=w1, in_=w1T[ft])
                ps = gps.tile([P, THALF], F32, tag="ps")
                for k in range(DT):
                    nc.tensor.matmul(ps, w1[:, k, :], x1T[:, k, gsl],
                                     start=(k == 0), stop=(k == DT - 1))
                if use_b1:
                    nc.scalar.activation(out=hT[:, ft, gsl], in_=ps,
                                         func=Relu,
                                         bias=b1_sb[:, ft:ft + 1], scale=1.0)
                else:
                    nc.scalar.activation(out=hT[:, ft, gsl], in_=ps,
                                         func=Relu)

        # interleave out_proj tiles 0..3 with tr/ao of batches 4..7
        for i in range(BHALF):
            emit_op_tile(i)
            emit_tr_ao(BHALF + i, attns[BHALF + i], v_sb1)
        st_attn.close()      # frees v_sb/bt/btT SBUF
        c2w = ExitStack()
        w1_pool = c2w.enter_context(tc.tile_pool(name="w1", bufs=2))
        emit_op_tile(4)
        emit_op_tile(5)
        emit_ffn1(0)
        emit_op_tile(6)
        emit_op_tile(7)
        emit_ffn1(1)
        c2w.close()      # frees w1 SBUF
        c2x.close()      # frees x1T SBUF

        # ---------------- phase D: FFN2 + LN2 ----------------
        with (
            tc.tile_pool(name="w2", bufs=1, side="right") as w2_pool,
            tc.tile_pool(name="ot", bufs=2, side="right") as ot_pool,
            tc.tile_pool(name="xr2", bufs=4, side="right") as xr2_pool,
        ):
            w2_tiles = []
            for c in range(NC_CH):
                w2 = w2_pool.tile([P, FT, CH], BF16, tag=f"w2{c}")
                nc.sync.dma_start(out=w2, in_=w2T[c])
                w2_tiles.append(w2)

            for ti in range(BL):
                tt = slice(ti * P, (ti + 1) * P)
                y2 = y_pool.tile([P, D], F32, tag="y")
                stats = stat_pool.tile([P, NC_CH, 6], F32, tag="stats")
                for c in range(NC_CH):
                    xr2 = xr2_pool.tile([P, CH], BF16, tag="xr2")
                    nc.sync.dma_start(
                        out=xr2, in_=x1_dram[ti * P:(ti + 1) * P,
                                             c * CH:(c + 1) * CH])
                    ps = gps.tile([P, CH], F32, tag="ps")
                    for k in range(FT):
                        nc.tensor.matmul(ps, hT[:, k, tt],
                                         w2_tiles[c][:, k, :],
                                         start=(k == 0), stop=(k == FT - 1))
                    csl = slice(c * CH, (c + 1) * CH)
                    nc.vector.tensor_add(out=y2[:, csl], in0=ps, in1=xr2)
                    if use_b2:
                        nc.vector.tensor_add(out=y2[:, csl], in0=y2[:, csl],
                                             in1=b2_sb[:, csl])
                    nc.vector.bn_stats(out=stats[:, c, :], in_=y2[:, csl])
                o_t = ot_pool.tile([P, D], F32, tag="o_t")
                emit_ln(y2, stats, eps2_sb, ln2g_sb, ln2b_sb, o_t,
                        ln2_affine)
                nc.sync.dma_start(out=out[ti * P:(ti + 1) * P, :], in_=o_t)

    nc.compile()
    return nc


def _prep_inputs(x, distances, mask, qkv_w, qkv_b, out_w, out_b,
                 bias_w1, bias_b1, bias_w2, bias_b2,
                 ffn_w1, ffn_b1, ffn_w2, ffn_b2,
                 ln1_g, ln1_b, ln2_g, ln2_b):
    """Host-side shard + weight formatting. Returns (flags, scales, in_maps).

    The learned distance-bias MLP adds a per-query bias broadcast over keys;
    softmax over keys is invariant to it, so it is skipped. The key-padding
    mask is applied multiplicatively on exp'd scores when non-trivial.
    """
    bf16 = ml_dtypes.bfloat16
    f8 = ml_dtypes.float8_e4m3fn
    perm = _qk_perm()

    x = np.asarray(x, np.float32)
    q_w = qkv_w[0:D][perm]
    k_w = qkv_w[D:2 * D][perm]
    v_w = qkv_w[2 * D:3 * D]

    sx = _po2(x)
    wqs = _po2(q_w)
    wks = _po2(k_w)
    wvs = _po2(v_w)
    wos = _po2(out_w)
    rscale = np.float32(AOS * wos)

    qk_w8 = _tileize(np.concatenate(
        [np.float32(wqs) * q_w, np.float32(wks) * k_w],
        axis=0).T.astype(f8), P)
    qk_wb = _tileize(np.concatenate(
        [np.float32(sx * wqs) * q_w, np.float32(sx * wks) * k_w],
        axis=0).T.astype(bf16), P)
    v_w8 = _tileize((np.float32(wvs) * v_w).T.astype(f8), CH)
    out_w8 = _tileize((np.float32(wos) * out_w).T.astype(f8), CH)
    w1T = _tileize(ffn_w1.T.astype(bf16), P)
    w2T = _tileize(ffn_w2.T.astype(bf16), CH)

    qk_b = np.concatenate([qkv_b[0:D][perm] * np.float32(1.0 / np.sqrt(HD)),
                           qkv_b[D:2 * D][perm]]).astype(np.float32)
    v_bv = np.ascontiguousarray(qkv_b[2 * D:3 * D]).astype(np.float32)

    flags = dict(
        use_qk_bias=bool(np.any(qk_b != 0)),
        use_v_bias=bool(np.any(v_bv != 0)),
        use_out_bias=bool(np.any(out_b != 0)),
        use_b1=bool(np.any(ffn_b1 != 0)),
        use_b2=bool(np.any(ffn_b2 != 0)),
        ln1_affine=not (np.all(ln1_g == 1) and np.all(ln1_b == 0)),
        ln2_affine=not (np.all(ln2_g == 1) and np.all(ln2_b == 0)),
        use_mask=not bool(np.all(mask)),
    )
    scales = dict(sx=sx, wqs=wqs, wks=wks, wvs=wvs, wos=wos)

    shared = {"qk_w8": qk_w8, "qk_wb": qk_wb, "v_w8": v_w8,
              "out_w8": out_w8, "w1T": w1T, "w2T": w2T}
    need_xtb = (QF8 < DT) or (KF8 < DT)
    if not ((QF8 > 0) or (KF8 > 0)):
        del shared["qk_w8"]
    if not need_xtb:
        del shared["qk_wb"]
    if flags["use_qk_bias"]:
        shared["qk_b"] = qk_b
    if flags["use_v_bias"]:
        shared["v_b"] = v_bv
    if flags["use_out_bias"]:
        shared["out_b"] = (rscale * out_b).astype(np.float32)
    if flags["use_b1"]:
        shared["b1"] = ffn_b1.astype(np.float32)
    if flags["use_b2"]:
        shared["b2"] = ffn_b2.astype(np.float32)
    if flags["ln1_affine"]:
        shared["ln1_g"] = ln1_g.astype(np.float32)
        shared["ln1_b"] = ln1_b.astype(np.float32)
    if flags["ln2_affine"]:
        shared["ln2_g"] = ln2_g.astype(np.float32)
        shared["ln2_b"] = ln2_b.astype(np.float32)

    in_maps = []
    for c in range(NCORES):
        xc = np.ascontiguousarray(x[c * BL:(c + 1) * BL].reshape(TOK, D))
        xcT = xc.T                             # [D, TOK]
        xT8_blocks = np.ascontiguousarray(
            (np.float32(sx) * xcT).reshape(DT, P, NHALF, THALF)
            .transpose(2, 1, 0, 3)).astype(f8)
        m = {"x_res": (rscale * xc).astype(np.float32),
             "xT8": xT8_blocks, **shared}
        if need_xtb:
            m["xTb"] = np.ascontiguousarray(
                xcT.reshape(DT, P, NHALF, THALF)
                .transpose(2, 1, 0, 3)).astype(bf16)
        if flags["use_mask"]:
            m["mask"] = mask[c * BL:(c + 1) * BL].astype(np.float32)
        in_maps.append(m)
    return flags, scales, in_maps


def run(trace=False, **inputs):
    """Build + run on 8 cores. Returns (output, BassKernelResults)."""
    from concourse.bass_utils import run_bass_kernel_spmd

    inputs = {k: np.asarray(v) for k, v in inputs.items()}
    flags, scales, in_maps = _prep_inputs(**inputs)
    nc = build_core_program(**flags, **scales)
    res = run_bass_kernel_spmd(nc, in_maps, list(range(NCORES)), trace=trace)
    out = np.stack([np.asarray(res.results[c]["out"], dtype=np.float32)
                    for c in range(NCORES)])
    return out.reshape(B, K, D), res


def kernel(**inputs):
    out, _ = run(trace=False, **inputs)
    return out


# revision 16
# speedup vs baseline: 1.4353x; 1.0065x over previous
"""NeighborAttentionLayer Trainium2 kernel (8-core data-parallel SPMD).

Strategy
--------
Data-parallel over B=64: each NeuronCore runs the full layer for 8 batches
(1024 tokens). No collectives.

v2: fp8-e4m3 DoubleRow matmuls (2 k-tiles contracted per PE pass) for the
V projection, out_proj, and optionally the Q/K projections (QF8/KF8 set how
many of the 20 contraction tiles run fp8; the bf16 remainder accumulates
into the same PSUM with host-matched scales). Scores / attn@V / FFN stay
bf16 for accuracy. attn-out is kept transposed in SBUF as fp8 (no DRAM
round-trip). out_proj -> LN1 -> FFN1 -> FFN2 -> LN2 run as one fused
per-token-tile pipeline: residual adds on DVE from PSUM, LN applied by the
scalar engine (Identity with per-partition bias/scale). Softmax skips
max-subtraction (logits bounded; exp in fp32). Per-batch attention work is
interleaved with dense GEMM chains so the PE never idles on softmax.

Scale bookkeeping: fp8 operands carry power-of-2 scales (SX on x, W*S on
weights, AOS on attn-out). The out_proj PSUM is AOS*WOS-scaled; the
residual x ships pre-scaled and LN1's eps is (AOS*WOS)^2-scaled, so
normalization absorbs the whole scale exactly.
"""

import numpy as np
import ml_dtypes
from contextlib import ExitStack

# ---- problem constants (hardcoded per contract) ----
B, K, D, H, DFF = 64, 128, 2560, 8, 1024
HD = D // H                    # 320
EPS = 1e-5
NCORES = 8
BL = B // NCORES               # 8 batches per core
TOK = BL * K                   # 1024 tokens per core
P = 128
DT = D // P                    # 20 d-tiles
FT = DFF // P                  # 8 dff-tiles
CH = 512                       # matmul moving-dim chunk (psum bank limit)
NHALF = 2
THALF = TOK // NHALF           # 512 tokens per half
BHALF = BL // NHALF            # 4 batches per half
QKT = 2 * DT                   # 40 q+k feature tiles
NC_CH = D // CH                # 5 output chunks of 512

# ---- fp8 knobs ----
QF8 = 20      # leading k-tiles (of 20) of the Q projection contracted in fp8
KF8 = 0       # same for K projection
AOS = 16.0    # fp8 scale for attn-out


def _qk_perm():
    """Head-pair interleaved feature order for q (and k) projections."""
    perm = []
    for p in range(H // 2):
        h0, h1 = 2 * p, 2 * p + 1
        perm.extend(range(HD * h0, HD * h0 + 256))         # tiles 5p+0, 5p+1
        perm.extend(range(HD * h0 + 256, HD * h0 + 320))   # tile 5p+2 lo
        perm.extend(range(HD * h1 + 256, HD * h1 + 320))   # tile 5p+2 hi
        perm.extend(range(HD * h1, HD * h1 + 256))         # tiles 5p+3, 5p+4
    return np.array(perm)


def _score_ktiles(h):
    """(tile, row0, row1) triples (within the 20 q-tiles) contracting head h."""
    p = h // 2
    if h % 2 == 0:
        return [(5 * p + 0, 0, 128), (5 * p + 1, 0, 128), (5 * p + 2, 0, 64)]
    return [(5 * p + 3, 0, 128), (5 * p + 4, 0, 128), (5 * p + 2, 64, 128)]


def _ao_segments():
    """Per d-tile (real feature order) segments for attn@V:
    list over tiles of [(head, d0, d1, psum_base), ...]."""
    segs = [[] for _ in range(DT)]
    for h in range(H):
        d = HD * h
        end = HD * (h + 1)
        while d < end:
            nxt = min(end, (d // P + 1) * P)
            segs[d // P].append((h, d, nxt, d % P))
            d = nxt
    return segs


def _tileize(wT, chunk):
    """[Kin, N] -> [N/chunk, 128, Kin/128, chunk] contiguous blocks."""
    kin, n = wT.shape
    ko = kin // P
    return np.ascontiguousarray(
        wT.reshape(ko, P, n // chunk, chunk).transpose(2, 1, 0, 3))


def _po2(a, target=224.0):
    m = float(np.abs(a).max())
    if m == 0.0:
        return 1.0
    return float(2.0 ** np.floor(np.log2(target / m)))


def build_core_program(use_qk_bias, use_v_bias, use_out_bias, use_b1, use_b2,
                       ln1_affine, ln2_affine, use_mask,
                       sx, wqs, wks, wvs, wos):
    import concourse.bass as bass
    import concourse.bacc as bacc
    import concourse.mybir as mybir
    import concourse.tile as tile
    from concourse.masks import make_identity

    F32 = mybir.dt.float32
    BF16 = mybir.dt.bfloat16
    F8 = mybir.dt.float8e4
    DR = mybir.MatmulPerfMode.DoubleRow

    need_xtb = (QF8 < DT) or (KF8 < DT)
    need_qk8 = (QF8 > 0) or (KF8 > 0)
    rscale = AOS * wos            # out_proj psum / residual scale

    nc = bacc.Bacc()
    dp = nc.declare_dram_parameter
    xT8 = dp("xT8", [NHALF, P, DT, THALF], F8, isOutput=False)
    xTb = dp("xTb", [NHALF, P, DT, THALF], BF16, isOutput=False) \
        if need_xtb else None
    qk_w8 = dp("qk_w8", [QKT, P, DT, P], F8, isOutput=False) \
        if need_qk8 else None
    qk_wb = dp("qk_wb", [QKT, P, DT, P], BF16, isOutput=False) \
        if need_xtb else None
    v_w8 = dp("v_w8", [NC_CH, P, DT, CH], F8, isOutput=False)
    out_w8 = dp("out_w8", [NC_CH, P, DT, CH], F8, isOutput=False)
    w1T = dp("w1T", [FT, P, DT, P], BF16, isOutput=False)
    w2T = dp("w2T", [NC_CH, P, FT, CH], BF16, isOutput=False)
    x_res = dp("x_res", [TOK, D], F32, isOutput=False)
    qk_b = dp("qk_b", [2 * D], F32, isOutput=False) if use_qk_bias else None
    v_b = dp("v_b", [D], F32, isOutput=False) if use_v_bias else None
    out_b = dp("out_b", [D], F32, isOutput=False) if use_out_bias else None
    b1 = dp("b1", [DFF], F32, isOutput=False) if use_b1 else None
    b2 = dp("b2", [D], F32, isOutput=False) if use_b2 else None
    ln1_g = dp("ln1_g", [D], F32, isOutput=False) if ln1_affine else None
    ln1_b = dp("ln1_b", [D], F32, isOutput=False) if ln1_affine else None
    ln2_g = dp("ln2_g", [D], F32, isOutput=False) if ln2_affine else None
    ln2_b = dp("ln2_b", [D], F32, isOutput=False) if ln2_affine else None
    mask_in = dp("mask", [BL, K], F32, isOutput=False) if use_mask else None
    out = dp("out", [TOK, D], F32, isOutput=True)

    x1_dram = nc.dram_tensor("x1_scratch", [TOK, D], BF16)

    Exp = mybir.ActivationFunctionType.Exp
    Relu = mybir.ActivationFunctionType.Relu
    Sqrt = mybir.ActivationFunctionType.Sqrt
    Copy = mybir.ActivationFunctionType.Copy
    Ident = mybir.ActivationFunctionType.Identity
    AX = mybir.AxisListType.X
    OP = mybir.AluOpType

    q_evac = 1.0 / (sx * wqs * float(np.sqrt(HD)))
    k_evac = 1.0 / (sx * wks)
    v_evac = 1.0 / (sx * wvs)

    def bcast_dram(ap_, n_part=P):
        return bass.AP(tensor=ap_.tensor, offset=ap_.offset,
                       ap=[[0, n_part]] + list(ap_.ap))

    ao_segs = _ao_segments()

    with tile.TileContext(nc) as tc, ExitStack() as st:
        consts = st.enter_context(tc.tile_pool(name="consts", bufs=1))
        persist = st.enter_context(tc.tile_pool(name="persist", bufs=1))
        # PSUM: 8 banks total, slots are bank-aligned.
        gps = st.enter_context(tc.tile_pool(name="gps", bufs=3, space="PSUM"))
        sps = st.enter_context(tc.tile_pool(name="sps", bufs=2, space="PSUM"))
        tps = st.enter_context(tc.tile_pool(name="tps", bufs=2, space="PSUM"))
        ops = st.enter_context(tc.tile_pool(name="ops", bufs=1, space="PSUM"))

        id_bf = consts.tile([P, P], BF16)
        make_identity(nc, id_bf)
        eps1_sb = consts.tile([P, 1], F32)
        nc.vector.memset(eps1_sb, rscale * rscale * EPS)
        eps2_sb = consts.tile([P, 1], F32)
        nc.vector.memset(eps2_sb, EPS)

        qkb_sb = None
        if use_qk_bias:
            qkb_sb = consts.tile([P, QKT], F32)
            nc.sync.dma_start(out=qkb_sb,
                              in_=qk_b[:].rearrange("(t p) -> p t", p=P))
        vb_sb = None
        if use_v_bias:
            vb_sb = consts.tile([P, D], F32)
            nc.gpsimd.dma_start(out=vb_sb, in_=bcast_dram(v_b[:]))
        outb_sb = None
        if use_out_bias:
            outb_sb = consts.tile([P, D], F32)
            nc.gpsimd.dma_start(out=outb_sb, in_=bcast_dram(out_b[:]))
        b1_sb = None
        if use_b1:
            b1_sb = consts.tile([P, FT], F32)
            nc.sync.dma_start(out=b1_sb,
                              in_=b1[:].rearrange("(t p) -> p t", p=P))
        b2_sb = None
        if use_b2:
            b2_sb = consts.tile([P, D], F32)
            nc.gpsimd.dma_start(out=b2_sb, in_=bcast_dram(b2[:]))
        ln1g_sb = ln1b_sb = ln2g_sb = ln2b_sb = None
        if ln1_affine:
            ln1g_sb = consts.tile([P, D], F32)
            nc.gpsimd.dma_start(out=ln1g_sb, in_=bcast_dram(ln1_g[:]))
            ln1b_sb = consts.tile([P, D], F32)
            nc.gpsimd.dma_start(out=ln1b_sb, in_=bcast_dram(ln1_b[:]))
        if ln2_affine:
            ln2g_sb = consts.tile([P, D], F32)
            nc.gpsimd.dma_start(out=ln2g_sb, in_=bcast_dram(ln2_g[:]))
            ln2b_sb = consts.tile([P, D], F32)
            nc.gpsimd.dma_start(out=ln2b_sb, in_=bcast_dram(ln2_b[:]))
        mask_sb = None
        if use_mask:
            mask_sb = consts.tile([P, BL, K], F32)
            nc.gpsimd.dma_start(out=mask_sb, in_=bcast_dram(mask_in[:, :]))

        # attn-out, transposed, fp8, SBUF-resident across phases
        aoT_sb = persist.tile([P, DT, TOK], F8)

        # -------------- attention pools (right side, close early) -------
        st_attn = ExitStack()
        attn_outer = st_attn.enter_context(
            tc.tile_pool(name="attn_outer", bufs=1, side="right"))
        bt = st_attn.enter_context(
            tc.tile_pool(name="bt", bufs=5, side="right"))
        btT = st_attn.enter_context(
            tc.tile_pool(name="btT", bufs=2, side="right"))

        qk_st = ExitStack()
        qkT_pool = qk_st.enter_context(
            tc.tile_pool(name="qkT", bufs=1, side="right"))
        qkT_sb = qkT_pool.tile([P, QKT, THALF], BF16, tag="qkT")

        proj_st = ExitStack()
        app = proj_st.enter_context(
            tc.tile_pool(name="attn_proj", bufs=1, side="right"))
        aw = proj_st.enter_context(
            tc.tile_pool(name="aw", bufs=2, side="right"))

        def dma_x8(half):
            x8 = app.tile([P, DT, THALF], F8, tag="x8", bufs=2)
            for s in range(4):
                nc.sync.dma_start(out=x8[:, 5 * s:5 * (s + 1), :],
                                  in_=xT8[half, :, 5 * s:5 * (s + 1), :])
            return x8

        def dma_xb(half):
            if not need_xtb:
                return None
            xb = app.tile([P, DT, THALF], BF16, tag="xb", bufs=1)
            for s in range(4):
                nc.sync.dma_start(out=xb[:, 5 * s:5 * (s + 1), :],
                                  in_=xTb[half, :, 5 * s:5 * (s + 1), :])
            return xb

        def emit_v_proj(xv, v_sb, c_range):
            for c in c_range:
                wv = aw.tile([P, DT, CH], F8, tag="wv")
                nc.sync.dma_start(out=wv, in_=v_w8[c])
                for t in range(BHALF):
                    ps = gps.tile([P, CH], F32, tag="ps")
                    for kp in range(DT // 2):
                        nc.tensor.matmul(
                            ps, xv[:, 2 * kp:2 * kp + 2, t * P:(t + 1) * P],
                            wv[:, 2 * kp:2 * kp + 2, :],
                            start=(kp == 0), stop=(kp == DT // 2 - 1),
                            perf_mode=DR)
                    osl = v_sb[:, t, c * CH:(c + 1) * CH]
                    nc.scalar.activation(out=osl, in_=ps, func=Copy,
                                         scale=v_evac)
                    if use_v_bias:
                        nc.vector.tensor_add(
                            out=osl, in0=osl,
                            in1=vb_sb[:, c * CH:(c + 1) * CH])

        def emit_qk_proj(x8, xb, j_range):
            for jt in j_range:
                f8n = QF8 if jt < DT else KF8
                evac = q_evac if jt < DT else k_evac
                w8t = wbt = None
                if f8n > 0:
                    w8t = aw.tile([P, f8n, P], F8, tag=f"w8_{f8n}")
                    nc.sync.dma_start(out=w8t, in_=qk_w8[jt][:, 0:f8n, :])
                if f8n < DT:
                    wbt = aw.tile([P, DT - f8n, P], BF16, tag=f"wb_{f8n}")
                    nc.sync.dma_start(out=wbt, in_=qk_wb[jt][:, f8n:DT, :])
                ps = gps.tile([P, CH], F32, tag="ps")
                n_mm = f8n // 2 + (DT - f8n)
                i = 0
                for kp in range(f8n // 2):
                    nc.tensor.matmul(
                        ps, w8t[:, 2 * kp:2 * kp + 2, :],
                        x8[:, 2 * kp:2 * kp + 2, :],
                        start=(i == 0), stop=(i == n_mm - 1), perf_mode=DR)
                    i += 1
                for k in range(f8n, DT):
                    nc.tensor.matmul(
                        ps, wbt[:, k - f8n, :], xb[:, k, :],
                        start=(i == 0), stop=(i == n_mm - 1))
                    i += 1
                if use_qk_bias:
                    nc.scalar.activation(out=qkT_sb[:, jt, :], in_=ps,
                                         func=Ident,
                                         bias=qkb_sb[:, jt:jt + 1],
                                         scale=evac)
                else:
                    nc.scalar.activation(out=qkT_sb[:, jt, :], in_=ps,
                                         func=Copy, scale=evac)

        def emit_scores_softmax(b):
            bi = b % BHALF
            csl = slice(bi * P, (bi + 1) * P)
            attn = bt.tile([P, H, P], BF16, tag="attn")
            esum = bt.tile([P, H], F32, tag="esum")
            rinv = bt.tile([P, H], F32, tag="rinv")
            for h in range(H):
                sc = sps.tile([P, P], F32, tag="sc")
                kts = _score_ktiles(h)
                for i, (t, r0, r1) in enumerate(kts):
                    nc.tensor.matmul(
                        sc, qkT_sb[r0:r1, t, csl],
                        qkT_sb[r0:r1, DT + t, csl],
                        start=(i == 0), stop=(i == len(kts) - 1))
                nc.scalar.activation(out=attn[:, h, :], in_=sc, func=Exp,
                                     accum_out=esum[:, h:h + 1])
                if use_mask:
                    nc.vector.tensor_mul(
                        out=attn[:, h, :], in0=attn[:, h, :],
                        in1=mask_sb[:, b, :])
                    nc.vector.tensor_reduce(
                        out=esum[:, h:h + 1], in_=attn[:, h, :],
                        axis=AX, op=OP.add)
                nc.vector.reciprocal(out=rinv[:, h:h + 1],
                                     in_=esum[:, h:h + 1])
                nc.vector.tensor_scalar_mul(
                    out=attn[:, h, :], in0=attn[:, h, :],
                    scalar1=rinv[:, h:h + 1])
            return attn

        def emit_tr_ao(b, attn, v_sb):
            bi = b % BHALF
            attnT = btT.tile([P, H, P], BF16, tag="attnT")
            for h in range(H):
                tp = tps.tile([P, P], BF16, tag="tp")
                nc.tensor.transpose(tp, attn[:, h, :], id_bf)
                nc.vector.tensor_copy(out=attnT[:, h, :], in_=tp)
            for t in range(DT):
                ao = ops.tile([P, P], F32, tag="ao")
                for (h, d0, d1, base) in ao_segs[t]:
                    w = d1 - d0
                    nc.tensor.matmul(
                        ao[base:base + w, :], v_sb[:, bi, d0:d1],
                        attnT[:, h, :], start=True, stop=True,
                        tile_position=((0, base) if base else None))
                nc.scalar.activation(
                    out=aoT_sb[:, t, b * P:(b + 1) * P], in_=ao,
                    func=Copy, scale=AOS)

        # ---- phase A emission: projections + batches 0..3 interleaved ----
        # tr/ao of batches 0..3 interleave with the half-1 Q/K GEMM chains
        # (not the V chains: v_sb is single-buffered, so its half-1 writers
        # must be emitted after every half-0 reader). Scores of batches 4..7
        # run right after QK h1 so the qkT pool can be released early.
        x8_0 = dma_x8(0)
        v_sb0 = attn_outer.tile([P, BHALF, D], BF16, tag="v")
        emit_v_proj(x8_0, v_sb0, range(NC_CH))
        xb_0 = dma_xb(0)
        emit_qk_proj(x8_0, xb_0, range(QKT))
        attns = {}
        for b in range(BHALF):
            attns[b] = emit_scores_softmax(b)
        x8_1 = dma_x8(1)
        xb_1 = dma_xb(1)
        for b in range(BHALF):
            emit_tr_ao(b, attns[b], v_sb0)
            emit_qk_proj(x8_1, xb_1, range(10 * b, 10 * (b + 1)))
        for b in range(BHALF, BL):
            attns[b] = emit_scores_softmax(b)
        v_sb1 = attn_outer.tile([P, BHALF, D], BF16, tag="v")
        emit_v_proj(x8_1, v_sb1, range(NC_CH))
        proj_st.close()      # frees xT8/xTb/aw SBUF
        qk_st.close()        # frees qkT SBUF (scores all emitted)

        # ------------- fused phase C pools (left side) -------------
        y_pool = st.enter_context(tc.tile_pool(name="y_pool", bufs=2))
        x1b_pool = st.enter_context(tc.tile_pool(name="x1b", bufs=2))
        xr_pool = st.enter_context(tc.tile_pool(name="xr", bufs=6))
        stat_pool = st.enter_context(tc.tile_pool(name="stat", bufs=2))
        wo_pool = st.enter_context(tc.tile_pool(name="wo", bufs=1))
        hT_pool = st.enter_context(tc.tile_pool(name="hT", bufs=1))
        hT = hT_pool.tile([P, FT, TOK], BF16)
        c2x = ExitStack()
        x1T_pool = c2x.enter_context(tc.tile_pool(name="x1T", bufs=1))
        x1T = x1T_pool.tile([P, DT, TOK], BF16)

        wo_tiles = []
        for c in range(NC_CH):
            wo = wo_pool.tile([P, DT, CH], F8, tag=f"wo{c}")
            nc.sync.dma_start(out=wo, in_=out_w8[c])
            wo_tiles.append(wo)

        def emit_ln(y_t, stats, eps_sb, g_sb, b_sb, out_t, affine):
            """LN over [P, D] given per-chunk bn stats; writes out_t."""
            mv = stat_pool.tile([P, 2], F32, tag="mv")
            nc.vector.bn_aggr(out=mv, in_=stats)
            std = stat_pool.tile([P, 1], F32, tag="std")
            nc.scalar.activation(out=std, in_=mv[:, 1:2], func=Sqrt,
                                 bias=eps_sb, scale=1.0)
            rstd = stat_pool.tile([P, 1], F32, tag="rstd")
            nc.vector.reciprocal(out=rstd, in_=std)
            nmr = stat_pool.tile([P, 1], F32, tag="nmr")
            nc.vector.tensor_scalar(out=nmr, in0=mv[:, 0:1], scalar1=rstd,
                                    scalar2=-1.0, op0=OP.mult, op1=OP.mult)
            nc.scalar.activation(out=out_t, in_=y_t, func=Ident,
                                 bias=nmr, scale=rstd)
            if affine:
                nc.vector.tensor_mul(out=out_t, in0=out_t, in1=g_sb)
                nc.vector.tensor_add(out=out_t, in0=out_t, in1=b_sb)

        def emit_op_tile(ti):
            """out_proj + residual + LN1 for token tile ti -> x1_dram."""
            tt = slice(ti * P, (ti + 1) * P)
            y_t = y_pool.tile([P, D], F32, tag="y")
            stats = stat_pool.tile([P, NC_CH, 6], F32, tag="stats")
            xrs = []
            for c in range(NC_CH):
                xr = xr_pool.tile([P, CH], F32, tag="xr")
                nc.sync.dma_start(
                    out=xr, in_=x_res[ti * P:(ti + 1) * P,
                                      c * CH:(c + 1) * CH])
                xrs.append(xr)
            for c in range(NC_CH):
                xr = xrs[c]
                ps = gps.tile([P, CH], F32, tag="ps")
                for kp in range(DT // 2):
                    nc.tensor.matmul(
                        ps, aoT_sb[:, 2 * kp:2 * kp + 2, tt],
                        wo_tiles[c][:, 2 * kp:2 * kp + 2, :],
                        start=(kp == 0), stop=(kp == DT // 2 - 1),
                        perf_mode=DR)
                csl = slice(c * CH, (c + 1) * CH)
                nc.vector.tensor_add(out=y_t[:, csl], in0=ps, in1=xr)
                if use_out_bias:
                    nc.vector.tensor_add(out=y_t[:, csl], in0=y_t[:, csl],
                                         in1=outb_sb[:, csl])
                nc.vector.bn_stats(out=stats[:, c, :], in_=y_t[:, csl])
            x1b = x1b_pool.tile([P, D], BF16, tag="x1b")
            emit_ln(y_t, stats, eps1_sb, ln1g_sb, ln1b_sb, x1b, ln1_affine)
            nc.sync.dma_start(out=x1_dram[ti * P:(ti + 1) * P, :], in_=x1b)
            # transpose x1 into x1T straight from SBUF
            for k in range(DT):
                tp = tps.tile([P, P], BF16, tag="tp")
                nc.tensor.transpose(tp, x1b[:, k * P:(k + 1) * P], id_bf)
                nc.vector.tensor_copy(
                    out=x1T[:, k, ti * P:(ti + 1) * P], in_=tp)

        def emit_ffn1(g):
            gsl = slice(g * THALF, (g + 1) * THALF)
            for ft in range(FT):
                w1 = w1_pool.tile([P, DT, P], BF16, tag="w1")
                nc.sync.dma_start(out=w1, in_=w1T[ft])
                ps = gps.tile([P, THALF], F32, tag="ps")
                for k in range(DT):
                    nc.tensor.matmul(ps, w1[:, k, :], x1T[:, k, gsl],
                                     start=(k == 0), stop=(k == DT - 1))
                if use_b1:
                    nc.scalar.activation(out=hT[:, ft, gsl], in_=ps,
                                         func=Relu,
                                         bias=b1_sb[:, ft:ft + 1], scale=1.0)
                else:
                    nc.scalar.activation(out=hT[:, ft, gsl], in_=ps,
                                         func=Relu)

        # interleave out_proj tiles 0..3 with tr/ao of batches 4..7
        for i in range(BHALF):
            emit_op_tile(i)
            emit_tr_ao(BHALF + i, attns[BHALF + i], v_sb1)
        st_attn.close()      # frees v_sb/bt/btT SBUF
        c2w = ExitStack()
        w1_pool = c2w.enter_context(tc.tile_pool(name="w1", bufs=2))
        emit_op_tile(4)
        emit_op_tile(5)
        emit_ffn1(0)
        emit_op_tile(6)
        emit_op_tile(7)
        emit_ffn1(1)
        c2w.close()      # frees w1 SBUF
        c2x.close()      # frees x1T SBUF

        # ---------------- phase D: FFN2 + LN2 ----------------
        with (
            tc.tile_pool(name="w2", bufs=1, side="right") as w2_pool,
            tc.tile_pool(name="ot", bufs=2, side="right") as ot_pool,
            tc.tile_pool(name="xr2", bufs=4, side="right") as xr2_pool,
        ):
            w2_tiles = []
            for c in range(NC_CH):
                w2 = w2_pool.tile([P, FT, CH], BF16, tag=f"w2{c}")
                nc.sync.dma_start(out=w2, in_=w2T[c])
                w2_tiles.append(w2)

            for ti in range(BL):
                tt = slice(ti * P, (ti + 1) * P)
                y2 = y_pool.tile([P, D], F32, tag="y")
                stats = stat_pool.tile([P, NC_CH, 6], F32, tag="stats")
                xr2s = []
                for c in range(NC_CH):
                    xr2 = xr2_pool.tile([P, CH], BF16, tag="xr2")
                    nc.sync.dma_start(
                        out=xr2, in_=x1_dram[ti * P:(ti + 1) * P,
                                             c * CH:(c + 1) * CH])
                    xr2s.append(xr2)
                for c in range(NC_CH):
                    xr2 = xr2s[c]
                    ps = gps.tile([P, CH], F32, tag="ps")
                    for k in range(FT):
                        nc.tensor.matmul(ps, hT[:, k, tt],
                                         w2_tiles[c][:, k, :],
                                         start=(k == 0), stop=(k == FT - 1))
                    csl = slice(c * CH, (c + 1) * CH)
                    nc.vector.tensor_add(out=y2[:, csl], in0=ps, in1=xr2)
                    if use_b2:
                        nc.vector.tensor_add(out=y2[:, csl], in0=y2[:, csl],
                                             in1=b2_sb[:, csl])
                    nc.vector.bn_stats(out=stats[:, c, :], in_=y2[:, csl])
                o_t = ot_pool.tile([P, D], F32, tag="o_t")
                emit_ln(y2, stats, eps2_sb, ln2g_sb, ln2b_sb, o_t,
                        ln2_affine)
                nc.sync.dma_start(out=out[ti * P:(ti + 1) * P, :], in_=o_t)

    nc.compile()
    return nc


def _prep_inputs(x, distances, mask, qkv_w, qkv_b, out_w, out_b,
                 bias_w1, bias_b1, bias_w2, bias_b2,
                 ffn_w1, ffn_b1, ffn_w2, ffn_b2,
                 ln1_g, ln1_b, ln2_g, ln2_b):
    """Host-side shard + weight formatting. Returns (flags, scales, in_maps).

    The learned distance-bias MLP adds a per-query bias broadcast over keys;
    softmax over keys is invariant to it, so it is skipped. The key-padding
    mask is applied multiplicatively on exp'd scores when non-trivial.
    """
    bf16 = ml_dtypes.bfloat16
    f8 = ml_dtypes.float8_e4m3fn
    perm = _qk_perm()

    x = np.asarray(x, np.float32)
    q_w = qkv_w[0:D][perm]
    k_w = qkv_w[D:2 * D][perm]
    v_w = qkv_w[2 * D:3 * D]

    sx = _po2(x)
    wqs = _po2(q_w)
    wks = _po2(k_w)
    wvs = _po2(v_w)
    wos = _po2(out_w)
    rscale = np.float32(AOS * wos)

    qk_w8 = _tileize(np.concatenate(
        [np.float32(wqs) * q_w, np.float32(wks) * k_w],
        axis=0).T.astype(f8), P)
    qk_wb = _tileize(np.concatenate(
        [np.float32(sx * wqs) * q_w, np.float32(sx * wks) * k_w],
        axis=0).T.astype(bf16), P)
    v_w8 = _tileize((np.float32(wvs) * v_w).T.astype(f8), CH)
    out_w8 = _tileize((np.float32(wos) * out_w).T.astype(f8), CH)
    w1T = _tileize(ffn_w1.T.astype(bf16), P)
    w2T = _tileize(ffn_w2.T.astype(bf16), CH)

    qk_b = np.concatenate([qkv_b[0:D][perm] * np.float32(1.0 / np.sqrt(HD)),
                           qkv_b[D:2 * D][perm]]).astype(np.float32)
    v_bv = np.ascontiguousarray(qkv_b[2 * D:3 * D]).astype(np.float32)

    flags = dict(
        use_qk_bias=bool(np.any(qk_b != 0)),
        use_v_bias=bool(np.any(v_bv != 0)),
        use_out_bias=bool(np.any(out_b != 0)),
        use_b1=bool(np.any(ffn_b1 != 0)),
        use_b2=bool(np.any(ffn_b2 != 0)),
        ln1_affine=not (np.all(ln1_g == 1) and np.all(ln1_b == 0)),
        ln2_affine=not (np.all(ln2_g == 1) and np.all(ln2_b == 0)),
        use_mask=not bool(np.all(mask)),
    )
    scales = dict(sx=sx, wqs=wqs, wks=wks, wvs=wvs, wos=wos)

    shared = {"qk_w8": qk_w8, "qk_wb": qk_wb, "v_w8": v_w8,
              "out_w8": out_w8, "w1T": w1T, "w2T": w2T}
    need_xtb = (QF8 < DT) or (KF8 < DT)
    if not ((QF8 > 0) or (KF8 > 0)):
        del shared["qk_w8"]
    if not need_xtb:
        del shared["qk_wb"]
    if flags["use_qk_bias"]:
        shared["qk_b"] = qk_b
    if flags["use_v_bias"]:
        shared["v_b"] = v_bv
    if flags["use_out_bias"]:
        shared["out_b"] = (rscale * out_b).astype(np.float32)
    if flags["use_b1"]:
        shared["b1"] = ffn_b1.astype(np.float32)
    if flags["use_b2"]:
        shared["b2"] = ffn_b2.astype(np.float32)
    if flags["ln1_affine"]:
        shared["ln1_g"] = ln1_g.astype(np.float32)
        shared["ln1_b"] = ln1_b.astype(np.float32)
    if flags["ln2_affine"]:
        shared["ln2_g"] = ln2_g.astype(np.float32)
        shared["ln2_b"] = ln2_b.astype(np.float32)

    in_maps = []
    for c in range(NCORES):
        xc = np.ascontiguousarray(x[c * BL:(c + 1) * BL].reshape(TOK, D))
        xcT = xc.T                             # [D, TOK]
        xT8_blocks = np.ascontiguousarray(
            (np.float32(sx) * xcT).reshape(DT, P, NHALF, THALF)
            .transpose(2, 1, 0, 3)).astype(f8)
        m = {"x_res": (rscale * xc).astype(np.float32),
             "xT8": xT8_blocks, **shared}
        if need_xtb:
            m["xTb"] = np.ascontiguousarray(
                xcT.reshape(DT, P, NHALF, THALF)
                .transpose(2, 1, 0, 3)).astype(bf16)
        if flags["use_mask"]:
            m["mask"] = mask[c * BL:(c + 1) * BL].astype(np.float32)
        in_maps.append(m)
    return flags, scales, in_maps


def run(trace=False, **inputs):
    """Build + run on 8 cores. Returns (output, BassKernelResults)."""
    from concourse.bass_utils import run_bass_kernel_spmd

    inputs = {k: np.asarray(v) for k, v in inputs.items()}
    flags, scales, in_maps = _prep_inputs(**inputs)
    nc = build_core_program(**flags, **scales)
    res = run_bass_kernel_spmd(nc, in_maps, list(range(NCORES)), trace=trace)
    out = np.stack([np.asarray(res.results[c]["out"], dtype=np.float32)
                    for c in range(NCORES)])
    return out.reshape(B, K, D), res


def kernel(**inputs):
    out, _ = run(trace=False, **inputs)
    return out


# revision 19
# speedup vs baseline: 1.4684x; 1.0231x over previous
"""NeighborAttentionLayer Trainium2 kernel (8-core data-parallel SPMD).

Strategy
--------
Data-parallel over B=64: each NeuronCore runs the full layer for 8 batches
(1024 tokens). No collectives.

v2: fp8-e4m3 DoubleRow matmuls (2 k-tiles contracted per PE pass) for the
V projection, out_proj, and optionally the Q/K projections (QF8/KF8 set how
many of the 20 contraction tiles run fp8; the bf16 remainder accumulates
into the same PSUM with host-matched scales). Scores / attn@V / FFN stay
bf16 for accuracy. attn-out is kept transposed in SBUF as fp8 (no DRAM
round-trip). out_proj -> LN1 -> FFN1 -> FFN2 -> LN2 run as one fused
per-token-tile pipeline: residual adds on DVE from PSUM, LN applied by the
scalar engine (Identity with per-partition bias/scale). Softmax skips
max-subtraction (logits bounded; exp in fp32). Per-batch attention work is
interleaved with dense GEMM chains so the PE never idles on softmax.

Scale bookkeeping: fp8 operands carry power-of-2 scales (SX on x, W*S on
weights, AOS on attn-out). The out_proj PSUM is AOS*WOS-scaled; the
residual x ships pre-scaled and LN1's eps is (AOS*WOS)^2-scaled, so
normalization absorbs the whole scale exactly.
"""

import numpy as np
import ml_dtypes
from contextlib import ExitStack

# ---- problem constants (hardcoded per contract) ----
B, K, D, H, DFF = 64, 128, 2560, 8, 1024
HD = D // H                    # 320
EPS = 1e-5
NCORES = 8
BL = B // NCORES               # 8 batches per core
TOK = BL * K                   # 1024 tokens per core
P = 128
DT = D // P                    # 20 d-tiles
FT = DFF // P                  # 8 dff-tiles
CH = 512                       # matmul moving-dim chunk (psum bank limit)
NHALF = 2
THALF = TOK // NHALF           # 512 tokens per half
BHALF = BL // NHALF            # 4 batches per half
QKT = 2 * DT                   # 40 q+k feature tiles
NC_CH = D // CH                # 5 output chunks of 512

# ---- fp8 knobs ----
QF8 = 20      # leading k-tiles (of 20) of the Q projection contracted in fp8
KF8 = 0       # same for K projection
AOS = 16.0    # fp8 scale for attn-out


def _qk_perm():
    """Head-pair interleaved feature order for q (and k) projections."""
    perm = []
    for p in range(H // 2):
        h0, h1 = 2 * p, 2 * p + 1
        perm.extend(range(HD * h0, HD * h0 + 256))         # tiles 5p+0, 5p+1
        perm.extend(range(HD * h0 + 256, HD * h0 + 320))   # tile 5p+2 lo
        perm.extend(range(HD * h1 + 256, HD * h1 + 320))   # tile 5p+2 hi
        perm.extend(range(HD * h1, HD * h1 + 256))         # tiles 5p+3, 5p+4
    return np.array(perm)


def _score_ktiles(h):
    """(tile, row0, row1) triples (within the 20 q-tiles) contracting head h."""
    p = h // 2
    if h % 2 == 0:
        return [(5 * p + 0, 0, 128), (5 * p + 1, 0, 128), (5 * p + 2, 0, 64)]
    return [(5 * p + 3, 0, 128), (5 * p + 4, 0, 128), (5 * p + 2, 64, 128)]


def _ao_segments():
    """Per d-tile (real feature order) segments for attn@V:
    list over tiles of [(head, d0, d1, psum_base), ...]."""
    segs = [[] for _ in range(DT)]
    for h in range(H):
        d = HD * h
        end = HD * (h + 1)
        while d < end:
            nxt = min(end, (d // P + 1) * P)
            segs[d // P].append((h, d, nxt, d % P))
            d = nxt
    return segs


def _tileize(wT, chunk):
    """[Kin, N] -> [N/chunk, 128, Kin/128, chunk] contiguous blocks."""
    kin, n = wT.shape
    ko = kin // P
    return np.ascontiguousarray(
        wT.reshape(ko, P, n // chunk, chunk).transpose(2, 1, 0, 3))


def _po2(a, target=224.0):
    m = float(np.abs(a).max())
    if m == 0.0:
        return 1.0
    return float(2.0 ** np.floor(np.log2(target / m)))


def build_core_program(use_qk_bias, use_v_bias, use_out_bias, use_b1, use_b2,
                       ln1_affine, ln2_affine, use_mask,
                       sx, wqs, wks, wvs, wos):
    import concourse.bass as bass
    import concourse.bacc as bacc
    import concourse.mybir as mybir
    import concourse.tile as tile
    from concourse.masks import make_identity

    F32 = mybir.dt.float32
    BF16 = mybir.dt.bfloat16
    F8 = mybir.dt.float8e4
    DR = mybir.MatmulPerfMode.DoubleRow

    need_xtb = (QF8 < DT) or (KF8 < DT)
    need_qk8 = (QF8 > 0) or (KF8 > 0)
    rscale = AOS * wos            # out_proj psum / residual scale

    nc = bacc.Bacc()
    dp = nc.declare_dram_parameter
    xT8 = dp("xT8", [NHALF, P, DT, THALF], F8, isOutput=False)
    xTb = dp("xTb", [NHALF, P, DT, THALF], BF16, isOutput=False) \
        if need_xtb else None
    qk_w8 = dp("qk_w8", [QKT, P, DT, P], F8, isOutput=False) \
        if need_qk8 else None
    qk_wb = dp("qk_wb", [QKT, P, DT, P], BF16, isOutput=False) \
        if need_xtb else None
    v_w8 = dp("v_w8", [NC_CH, P, DT, CH], F8, isOutput=False)
    out_w8 = dp("out_w8", [NC_CH, P, DT, CH], F8, isOutput=False)
    w1T = dp("w1T", [FT, P, DT, P], BF16, isOutput=False)
    w2T = dp("w2T", [NC_CH, P, FT, CH], BF16, isOutput=False)
    x_res = dp("x_res", [TOK, D], F32, isOutput=False)
    qk_b = dp("qk_b", [2 * D], F32, isOutput=False) if use_qk_bias else None
    v_b = dp("v_b", [D], F32, isOutput=False) if use_v_bias else None
    out_b = dp("out_b", [D], F32, isOutput=False) if use_out_bias else None
    b1 = dp("b1", [DFF], F32, isOutput=False) if use_b1 else None
    b2 = dp("b2", [D], F32, isOutput=False) if use_b2 else None
    ln1_g = dp("ln1_g", [D], F32, isOutput=False) if ln1_affine else None
    ln1_b = dp("ln1_b", [D], F32, isOutput=False) if ln1_affine else None
    ln2_g = dp("ln2_g", [D], F32, isOutput=False) if ln2_affine else None
    ln2_b = dp("ln2_b", [D], F32, isOutput=False) if ln2_affine else None
    mask_in = dp("mask", [BL, K], F32, isOutput=False) if use_mask else None
    out = dp("out", [TOK, D], F32, isOutput=True)

    x1_dram = nc.dram_tensor("x1_scratch", [TOK, D], BF16)

    Exp = mybir.ActivationFunctionType.Exp
    Relu = mybir.ActivationFunctionType.Relu
    Sqrt = mybir.ActivationFunctionType.Sqrt
    Copy = mybir.ActivationFunctionType.Copy
    Ident = mybir.ActivationFunctionType.Identity
    AX = mybir.AxisListType.X
    OP = mybir.AluOpType

    q_evac = 1.0 / (sx * wqs * float(np.sqrt(HD)))
    k_evac = 1.0 / (sx * wks)
    v_evac = 1.0 / (sx * wvs)

    def bcast_dram(ap_, n_part=P):
        return bass.AP(tensor=ap_.tensor, offset=ap_.offset,
                       ap=[[0, n_part]] + list(ap_.ap))

    ao_segs = _ao_segments()

    with tile.TileContext(nc) as tc, ExitStack() as st:
        consts = st.enter_context(tc.tile_pool(name="consts", bufs=1))
        persist = st.enter_context(tc.tile_pool(name="persist", bufs=1))
        # PSUM: 8 banks total, slots are bank-aligned.
        gps = st.enter_context(tc.tile_pool(name="gps", bufs=3, space="PSUM"))
        sps = st.enter_context(tc.tile_pool(name="sps", bufs=2, space="PSUM"))
        tps = st.enter_context(tc.tile_pool(name="tps", bufs=2, space="PSUM"))
        ops = st.enter_context(tc.tile_pool(name="ops", bufs=1, space="PSUM"))

        id_bf = consts.tile([P, P], BF16)
        make_identity(nc, id_bf)
        eps1_sb = consts.tile([P, 1], F32)
        nc.vector.memset(eps1_sb, rscale * rscale * EPS)
        eps2_sb = consts.tile([P, 1], F32)
        nc.vector.memset(eps2_sb, EPS)

        qkb_sb = None
        if use_qk_bias:
            qkb_sb = consts.tile([P, QKT], F32)
            nc.sync.dma_start(out=qkb_sb,
                              in_=qk_b[:].rearrange("(t p) -> p t", p=P))
        vb_sb = None
        if use_v_bias:
            vb_sb = consts.tile([P, D], F32)
            nc.gpsimd.dma_start(out=vb_sb, in_=bcast_dram(v_b[:]))
        outb_sb = None
        if use_out_bias:
            outb_sb = consts.tile([P, D], F32)
            nc.gpsimd.dma_start(out=outb_sb, in_=bcast_dram(out_b[:]))
        b1_sb = None
        if use_b1:
            b1_sb = consts.tile([P, FT], F32)
            nc.sync.dma_start(out=b1_sb,
                              in_=b1[:].rearrange("(t p) -> p t", p=P))
        b2_sb = None
        if use_b2:
            b2_sb = consts.tile([P, D], F32)
            nc.gpsimd.dma_start(out=b2_sb, in_=bcast_dram(b2[:]))
        ln1g_sb = ln1b_sb = ln2g_sb = ln2b_sb = None
        if ln1_affine:
            ln1g_sb = consts.tile([P, D], F32)
            nc.gpsimd.dma_start(out=ln1g_sb, in_=bcast_dram(ln1_g[:]))
            ln1b_sb = consts.tile([P, D], F32)
            nc.gpsimd.dma_start(out=ln1b_sb, in_=bcast_dram(ln1_b[:]))
        if ln2_affine:
            ln2g_sb = consts.tile([P, D], F32)
            nc.gpsimd.dma_start(out=ln2g_sb, in_=bcast_dram(ln2_g[:]))
            ln2b_sb = consts.tile([P, D], F32)
            nc.gpsimd.dma_start(out=ln2b_sb, in_=bcast_dram(ln2_b[:]))
        mask_sb = None
        if use_mask:
            mask_sb = consts.tile([P, BL, K], F32)
            nc.gpsimd.dma_start(out=mask_sb, in_=bcast_dram(mask_in[:, :]))

        # attn-out, transposed, fp8, SBUF-resident across phases
        aoT_sb = persist.tile([P, DT, TOK], F8)

        # -------------- attention pools (right side, close early) -------
        st_attn = ExitStack()
        bt = st_attn.enter_context(
            tc.tile_pool(name="bt", bufs=5, side="right"))
        btT = st_attn.enter_context(
            tc.tile_pool(name="btT", bufs=2, side="right"))
        v1_pool = st_attn.enter_context(
            tc.tile_pool(name="v1", bufs=1, side="right"))

        v0_st = ExitStack()
        v0_pool = v0_st.enter_context(
            tc.tile_pool(name="v0", bufs=1, side="right"))

        qk_st = ExitStack()
        qkT_pool = qk_st.enter_context(
            tc.tile_pool(name="qkT", bufs=1, side="right"))
        qkT_sb = qkT_pool.tile([P, QKT, THALF], BF16, tag="qkT")

        proj_st = ExitStack()
        app = proj_st.enter_context(
            tc.tile_pool(name="attn_proj", bufs=1, side="right"))
        aw = proj_st.enter_context(
            tc.tile_pool(name="aw", bufs=2, side="right"))

        def dma_x8(half):
            x8 = app.tile([P, DT, THALF], F8, tag="x8", bufs=2)
            for s in range(4):
                nc.sync.dma_start(out=x8[:, 5 * s:5 * (s + 1), :],
                                  in_=xT8[half, :, 5 * s:5 * (s + 1), :])
            return x8

        def dma_xb(half):
            if not need_xtb:
                return None
            xb = app.tile([P, DT, THALF], BF16, tag="xb", bufs=1)
            for s in range(4):
                nc.sync.dma_start(out=xb[:, 5 * s:5 * (s + 1), :],
                                  in_=xTb[half, :, 5 * s:5 * (s + 1), :])
            return xb

        def emit_v_proj(xv, v_sb, c_range):
            for c in c_range:
                wv = aw.tile([P, DT, CH], F8, tag="wv")
                nc.sync.dma_start(out=wv, in_=v_w8[c])
                for t in range(BHALF):
                    ps = gps.tile([P, CH], F32, tag="ps")
                    for kp in range(DT // 2):
                        nc.tensor.matmul(
                            ps, xv[:, 2 * kp:2 * kp + 2, t * P:(t + 1) * P],
                            wv[:, 2 * kp:2 * kp + 2, :],
                            start=(kp == 0), stop=(kp == DT // 2 - 1),
                            perf_mode=DR)
                    osl = v_sb[:, t, c * CH:(c + 1) * CH]
                    nc.scalar.activation(out=osl, in_=ps, func=Copy,
                                         scale=v_evac)
                    if use_v_bias:
                        nc.vector.tensor_add(
                            out=osl, in0=osl,
                            in1=vb_sb[:, c * CH:(c + 1) * CH])

        def emit_qk_proj(x8, xb, j_range):
            for jt in j_range:
                f8n = QF8 if jt < DT else KF8
                evac = q_evac if jt < DT else k_evac
                w8t = wbt = None
                if f8n > 0:
                    w8t = aw.tile([P, f8n, P], F8, tag=f"w8_{f8n}")
                    nc.sync.dma_start(out=w8t, in_=qk_w8[jt][:, 0:f8n, :])
                if f8n < DT:
                    wbt = aw.tile([P, DT - f8n, P], BF16, tag=f"wb_{f8n}")
                    nc.sync.dma_start(out=wbt, in_=qk_wb[jt][:, f8n:DT, :])
                ps = gps.tile([P, CH], F32, tag="ps")
                n_mm = f8n // 2 + (DT - f8n)
                i = 0
                for kp in range(f8n // 2):
                    nc.tensor.matmul(
                        ps, w8t[:, 2 * kp:2 * kp + 2, :],
                        x8[:, 2 * kp:2 * kp + 2, :],
                        start=(i == 0), stop=(i == n_mm - 1), perf_mode=DR)
                    i += 1
                for k in range(f8n, DT):
                    nc.tensor.matmul(
                        ps, wbt[:, k - f8n, :], xb[:, k, :],
                        start=(i == 0), stop=(i == n_mm - 1))
                    i += 1
                if use_qk_bias:
                    nc.scalar.activation(out=qkT_sb[:, jt, :], in_=ps,
                                         func=Ident,
                                         bias=qkb_sb[:, jt:jt + 1],
                                         scale=evac)
                else:
                    nc.scalar.activation(out=qkT_sb[:, jt, :], in_=ps,
                                         func=Copy, scale=evac)

        def emit_scores_softmax(b):
            bi = b % BHALF
            csl = slice(bi * P, (bi + 1) * P)
            attn = bt.tile([P, H, P], BF16, tag="attn")
            esum = bt.tile([P, H], F32, tag="esum")
            rinv = bt.tile([P, H], F32, tag="rinv")
            for h in range(H):
                sc = sps.tile([P, P], F32, tag="sc")
                kts = _score_ktiles(h)
                for i, (t, r0, r1) in enumerate(kts):
                    nc.tensor.matmul(
                        sc, qkT_sb[r0:r1, t, csl],
                        qkT_sb[r0:r1, DT + t, csl],
                        start=(i == 0), stop=(i == len(kts) - 1))
                nc.scalar.activation(out=attn[:, h, :], in_=sc, func=Exp,
                                     accum_out=esum[:, h:h + 1])
                if use_mask:
                    nc.vector.tensor_mul(
                        out=attn[:, h, :], in0=attn[:, h, :],
                        in1=mask_sb[:, b, :])
                    nc.vector.tensor_reduce(
                        out=esum[:, h:h + 1], in_=attn[:, h, :],
                        axis=AX, op=OP.add)
                nc.vector.reciprocal(out=rinv[:, h:h + 1],
                                     in_=esum[:, h:h + 1])
                nc.vector.tensor_scalar_mul(
                    out=attn[:, h, :], in0=attn[:, h, :],
                    scalar1=rinv[:, h:h + 1])
            return attn

        def emit_tr_ao(b, attn, v_sb):
            bi = b % BHALF
            attnT = btT.tile([P, H, P], BF16, tag="attnT")
            for h in range(H):
                tp = tps.tile([P, P], BF16, tag="tp")
                nc.tensor.transpose(tp, attn[:, h, :], id_bf)
                nc.vector.tensor_copy(out=attnT[:, h, :], in_=tp)
            for t in range(DT):
                ao = ops.tile([P, P], F32, tag="ao")
                for (h, d0, d1, base) in ao_segs[t]:
                    w = d1 - d0
                    nc.tensor.matmul(
                        ao[base:base + w, :], v_sb[:, bi, d0:d1],
                        attnT[:, h, :], start=True, stop=True,
                        tile_position=((0, base) if base else None))
                nc.scalar.activation(
                    out=aoT_sb[:, t, b * P:(b + 1) * P], in_=ao,
                    func=Copy, scale=AOS)

        # ---- phase A emission: projections + batches 0..3 interleaved ----
        # scores b0..3 interleave with the half-1 V chains (separate v1
        # buffer, so no WAR on v0); tr/ao b0..3 interleave with the half-1
        # Q/K chains (safe: every h0 score read is emitted first). Scores
        # b4..7 run right after QK h1 so qkT can be released early.
        x8_0 = dma_x8(0)
        v_sb0 = v0_pool.tile([P, BHALF, D], BF16, tag="v0")
        emit_v_proj(x8_0, v_sb0, range(NC_CH))
        xb_0 = dma_xb(0)
        emit_qk_proj(x8_0, xb_0, range(QKT))
        x8_1 = dma_x8(1)
        xb_1 = dma_xb(1)
        v_sb1 = v1_pool.tile([P, BHALF, D], BF16, tag="v1")
        attns = {}
        for b in range(BHALF):
            attns[b] = emit_scores_softmax(b)
            emit_v_proj(x8_1, v_sb1, range(b, b + 1))
        emit_v_proj(x8_1, v_sb1, range(BHALF, NC_CH))
        for b in range(BHALF):
            emit_tr_ao(b, attns[b], v_sb0)
            emit_qk_proj(x8_1, xb_1, range(10 * b, 10 * (b + 1)))
        for b in range(BHALF, BL):
            attns[b] = emit_scores_softmax(b)
        proj_st.close()      # frees xT8/xTb/aw SBUF
        qk_st.close()        # frees qkT SBUF (scores all emitted)
        v0_st.close()        # frees v0 SBUF (tr/ao b0..3 emitted)

        # ------------- fused phase C pools (left side) -------------
        y_pool = st.enter_context(tc.tile_pool(name="y_pool", bufs=2))
        x1b_pool = st.enter_context(tc.tile_pool(name="x1b", bufs=2))
        xr_pool = st.enter_context(tc.tile_pool(name="xr", bufs=6))
        stat_pool = st.enter_context(tc.tile_pool(name="stat", bufs=2))
        wo_pool = st.enter_context(tc.tile_pool(name="wo", bufs=1))
        hT_pool = st.enter_context(tc.tile_pool(name="hT", bufs=1))
        hT = hT_pool.tile([P, FT, TOK], BF16)
        c2x = ExitStack()
        x1T_pool = c2x.enter_context(tc.tile_pool(name="x1T", bufs=1))
        x1T = x1T_pool.tile([P, DT, TOK], BF16)

        wo_tiles = []
        for c in range(NC_CH):
            wo = wo_pool.tile([P, DT, CH], F8, tag=f"wo{c}")
            nc.sync.dma_start(out=wo, in_=out_w8[c])
            wo_tiles.append(wo)

        def emit_ln(y_t, stats, eps_sb, g_sb, b_sb, out_t, affine):
            """LN over [P, D] given per-chunk bn stats; writes out_t."""
            mv = stat_pool.tile([P, 2], F32, tag="mv")
            nc.vector.bn_aggr(out=mv, in_=stats)
            std = stat_pool.tile([P, 1], F32, tag="std")
            nc.scalar.activation(out=std, in_=mv[:, 1:2], func=Sqrt,
                                 bias=eps_sb, scale=1.0)
            rstd = stat_pool.tile([P, 1], F32, tag="rstd")
            nc.vector.reciprocal(out=rstd, in_=std)
            nmr = stat_pool.tile([P, 1], F32, tag="nmr")
            nc.vector.tensor_scalar(out=nmr, in0=mv[:, 0:1], scalar1=rstd,
                                    scalar2=-1.0, op0=OP.mult, op1=OP.mult)
            nc.scalar.activation(out=out_t, in_=y_t, func=Ident,
                                 bias=nmr, scale=rstd)
            if affine:
                nc.vector.tensor_mul(out=out_t, in0=out_t, in1=g_sb)
                nc.vector.tensor_add(out=out_t, in0=out_t, in1=b_sb)

        def emit_op_tile(ti):
            """out_proj + residual + LN1 for token tile ti -> x1_dram."""
            tt = slice(ti * P, (ti + 1) * P)
            y_t = y_pool.tile([P, D], F32, tag="y")
            stats = stat_pool.tile([P, NC_CH, 6], F32, tag="stats")
            xrs = []
            for c in range(NC_CH):
                xr = xr_pool.tile([P, CH], F32, tag="xr")
                nc.sync.dma_start(
                    out=xr, in_=x_res[ti * P:(ti + 1) * P,
                                      c * CH:(c + 1) * CH])
                xrs.append(xr)
            for c in range(NC_CH):
                xr = xrs[c]
                ps = gps.tile([P, CH], F32, tag="ps")
                for kp in range(DT // 2):
                    nc.tensor.matmul(
                        ps, aoT_sb[:, 2 * kp:2 * kp + 2, tt],
                        wo_tiles[c][:, 2 * kp:2 * kp + 2, :],
                        start=(kp == 0), stop=(kp == DT // 2 - 1),
                        perf_mode=DR)
                csl = slice(c * CH, (c + 1) * CH)
                nc.vector.tensor_add(out=y_t[:, csl], in0=ps, in1=xr)
                if use_out_bias:
                    nc.vector.tensor_add(out=y_t[:, csl], in0=y_t[:, csl],
                                         in1=outb_sb[:, csl])
                nc.vector.bn_stats(out=stats[:, c, :], in_=y_t[:, csl])
            x1b = x1b_pool.tile([P, D], BF16, tag="x1b")
            emit_ln(y_t, stats, eps1_sb, ln1g_sb, ln1b_sb, x1b, ln1_affine)
            nc.sync.dma_start(out=x1_dram[ti * P:(ti + 1) * P, :], in_=x1b)
            # transpose x1 into x1T straight from SBUF
            for k in range(DT):
                tp = tps.tile([P, P], BF16, tag="tp")
                nc.tensor.transpose(tp, x1b[:, k * P:(k + 1) * P], id_bf)
                nc.vector.tensor_copy(
                    out=x1T[:, k, ti * P:(ti + 1) * P], in_=tp)

        def emit_ffn1(g):
            gsl = slice(g * THALF, (g + 1) * THALF)
            for ft in range(FT):
                w1 = w1_pool.tile([P, DT, P], BF16, tag="w1")
                nc.sync.dma_start(out=w1, in_=w1T[ft])
                ps = gps.tile([P, THALF], F32, tag="ps")
                for k in range(DT):
                    nc.tensor.matmul(ps, w1[:, k, :], x1T[:, k, gsl],
                                     start=(k == 0), stop=(k == DT - 1))
                if use_b1:
                    nc.scalar.activation(out=hT[:, ft, gsl], in_=ps,
                                         func=Relu,
                                         bias=b1_sb[:, ft:ft + 1], scale=1.0)
                else:
                    nc.scalar.activation(out=hT[:, ft, gsl], in_=ps,
                                         func=Relu)

        # interleave out_proj tiles 0..3 with tr/ao of batches 4..7
        for i in range(BHALF):
            emit_op_tile(i)
            emit_tr_ao(BHALF + i, attns[BHALF + i], v_sb1)
        st_attn.close()      # frees v_sb/bt/btT SBUF
        c2w = ExitStack()
        w1_pool = c2w.enter_context(tc.tile_pool(name="w1", bufs=2))
        emit_op_tile(4)
        emit_op_tile(5)
        emit_ffn1(0)
        emit_op_tile(6)
        emit_op_tile(7)
        emit_ffn1(1)
        c2w.close()      # frees w1 SBUF
        c2x.close()      # frees x1T SBUF

        # ---------------- phase D: FFN2 + LN2 ----------------
        with (
            tc.tile_pool(name="w2", bufs=1, side="right") as w2_pool,
            tc.tile_pool(name="ot", bufs=2, side="right") as ot_pool,
            tc.tile_pool(name="xr2", bufs=4, side="right") as xr2_pool,
        ):
            w2_tiles = []
            for c in range(NC_CH):
                w2 = w2_pool.tile([P, FT, CH], BF16, tag=f"w2{c}")
                nc.sync.dma_start(out=w2, in_=w2T[c])
                w2_tiles.append(w2)

            for ti in range(BL):
                tt = slice(ti * P, (ti + 1) * P)
                y2 = y_pool.tile([P, D], F32, tag="y")
                stats = stat_pool.tile([P, NC_CH, 6], F32, tag="stats")
                xr2s = []
                for c in range(NC_CH):
                    xr2 = xr2_pool.tile([P, CH], BF16, tag="xr2")
                    nc.sync.dma_start(
                        out=xr2, in_=x1_dram[ti * P:(ti + 1) * P,
                                             c * CH:(c + 1) * CH])
                    xr2s.append(xr2)
                for c in range(NC_CH):
                    xr2 = xr2s[c]
                    ps = gps.tile([P, CH], F32, tag="ps")
                    for k in range(FT):
                        nc.tensor.matmul(ps, hT[:, k, tt],
                                         w2_tiles[c][:, k, :],
                                         start=(k == 0), stop=(k == FT - 1))
                    csl = slice(c * CH, (c + 1) * CH)
                    nc.vector.tensor_add(out=y2[:, csl], in0=ps, in1=xr2)
                    if use_b2:
                        nc.vector.tensor_add(out=y2[:, csl], in0=y2[:, csl],
                                             in1=b2_sb[:, csl])
                    nc.vector.bn_stats(out=stats[:, c, :], in_=y2[:, csl])
                o_t = ot_pool.tile([P, D], F32, tag="o_t")
                emit_ln(y2, stats, eps2_sb, ln2g_sb, ln2b_sb, o_t,
                        ln2_affine)
                nc.sync.dma_start(out=out[ti * P:(ti + 1) * P, :], in_=o_t)

    nc.compile()
    return nc


def _prep_inputs(x, distances, mask, qkv_w, qkv_b, out_w, out_b,
                 bias_w1, bias_b1, bias_w2, bias_b2,
                 ffn_w1, ffn_b1, ffn_w2, ffn_b2,
                 ln1_g, ln1_b, ln2_g, ln2_b):
    """Host-side shard + weight formatting. Returns (flags, scales, in_maps).

    The learned distance-bias MLP adds a per-query bias broadcast over keys;
    softmax over keys is invariant to it, so it is skipped. The key-padding
    mask is applied multiplicatively on exp'd scores when non-trivial.
    """
    bf16 = ml_dtypes.bfloat16
    f8 = ml_dtypes.float8_e4m3fn
    perm = _qk_perm()

    x = np.asarray(x, np.float32)
    q_w = qkv_w[0:D][perm]
    k_w = qkv_w[D:2 * D][perm]
    v_w = qkv_w[2 * D:3 * D]

    sx = _po2(x)
    wqs = _po2(q_w)
    wks = _po2(k_w)
    wvs = _po2(v_w)
    wos = _po2(out_w)
    rscale = np.float32(AOS * wos)

    qk_w8 = _tileize(np.concatenate(
        [np.float32(wqs) * q_w, np.float32(wks) * k_w],
        axis=0).T.astype(f8), P)
    qk_wb = _tileize(np.concatenate(
        [np.float32(sx * wqs) * q_w, np.float32(sx * wks) * k_w],
        axis=0).T.astype(bf16), P)
    v_w8 = _tileize((np.float32(wvs) * v_w).T.astype(f8), CH)
    out_w8 = _tileize((np.float32(wos) * out_w).T.astype(f8), CH)
    w1T = _tileize(ffn_w1.T.astype(bf16), P)
    w2T = _tileize(ffn_w2.T.astype(bf16), CH)

    qk_b = np.concatenate([qkv_b[0:D][perm] * np.float32(1.0 / np.sqrt(HD)),
                           qkv_b[D:2 * D][perm]]).astype(np.float32)
    v_bv = np.ascontiguousarray(qkv_b[2 * D:3 * D]).astype(np.float32)

    flags = dict(
        use_qk_bias=bool(np.any(qk_b != 0)),
        use_v_bias=bool(np.any(v_bv != 0)),
        use_out_bias=bool(np.any(out_b != 0)),
        use_b1=bool(np.any(ffn_b1 != 0)),
        use_b2=bool(np.any(ffn_b2 != 0)),
        ln1_affine=not (np.all(ln1_g == 1) and np.all(ln1_b == 0)),
        ln2_affine=not (np.all(ln2_g == 1) and np.all(ln2_b == 0)),
        use_mask=not bool(np.all(mask)),
    )
    scales = dict(sx=sx, wqs=wqs, wks=wks, wvs=wvs, wos=wos)

    shared = {"qk_w8": qk_w8, "qk_wb": qk_wb, "v_w8": v_w8,
              "out_w8": out_w8, "w1T": w1T, "w2T": w2T}
    need_xtb = (QF8 < DT) or (KF8 < DT)
    if not ((QF8 > 0) or (KF8 > 0)):
        del shared["qk_w8"]
    if not need_xtb:
        del shared["qk_wb"]
    if flags["use_qk_bias"]:
        shared["qk_b"] = qk_b
    if flags["use_v_bias"]:
        shared["v_b"] = v_bv
    if flags["use_out_bias"]:
        shared["out_b"] = (rscale * out_b).astype(np.float32)
    if flags["use_b1"]:
        shared["b1"] = ffn_b1.astype(np.float32)
    if flags["use_b2"]:
        shared["b2"] = ffn_b2.astype(np.float32)
    if flags["ln1_affine"]:
        shared["ln1_g"] = ln1_g.astype(np.float32)
        shared["ln1_b"] = ln1_b.astype(np.float32)
    if flags["ln2_affine"]:
        shared["ln2_g"] = ln2_g.astype(np.float32)
        shared["ln2_b"] = ln2_b.astype(np.float32)

    in_maps = []
    for c in range(NCORES):
        xc = np.ascontiguousarray(x[c * BL:(c + 1) * BL].reshape(TOK, D))
        xcT = xc.T                             # [D, TOK]
        xT8_blocks = np.ascontiguousarray(
            (np.float32(sx) * xcT).reshape(DT, P, NHALF, THALF)
            .transpose(2, 1, 0, 3)).astype(f8)
        m = {"x_res": (rscale * xc).astype(np.float32),
             "xT8": xT8_blocks, **shared}
        if need_xtb:
            m["xTb"] = np.ascontiguousarray(
                xcT.reshape(DT, P, NHALF, THALF)
                .transpose(2, 1, 0, 3)).astype(bf16)
        if flags["use_mask"]:
            m["mask"] = mask[c * BL:(c + 1) * BL].astype(np.float32)
        in_maps.append(m)
    return flags, scales, in_maps


def run(trace=False, **inputs):
    """Build + run on 8 cores. Returns (output, BassKernelResults)."""
    from concourse.bass_utils import run_bass_kernel_spmd

    inputs = {k: np.asarray(v) for k, v in inputs.items()}
    flags, scales, in_maps = _prep_inputs(**inputs)
    nc = build_core_program(**flags, **scales)
    res = run_bass_kernel_spmd(nc, in_maps, list(range(NCORES)), trace=trace)
    out = np.stack([np.asarray(res.results[c]["out"], dtype=np.float32)
                    for c in range(NCORES)])
    return out.reshape(B, K, D), res


def kernel(**inputs):
    out, _ = run(trace=False, **inputs)
    return out
